# revision 1
# baseline (speedup 1.0000x reference)
"""Hawk (RG-LRU) block kernel for Trainium2, SPMD over 8 NeuronCores.

Sharding: tokens. Core k handles batch b=k//2, half h=k%2 (2048 tokens).
All weights replicated (host-transposed into matmul-ready layouts).
On-chip layout is channel-major [channel partitions, time free]; the
diagonal recurrence runs as hardware tensor_tensor_scan along the free
dim. The cross-half scan carry moves via a pairwise AllReduce of 4KB.
Matmuls run in float32r (full-rate, ~1.5e-4 rel err).
"""
import sys

sys.path.insert(0, "/opt/trn_rl_repo")

import numpy as np
from contextlib import ExitStack

import concourse.bass as bass
import concourse.tile as tile
import concourse.bacc as bacc
from concourse import mybir
from concourse.bass_utils import run_bass_kernel_spmd

F32 = mybir.dt.float32
F32R = mybir.dt.float32r
BF16 = mybir.dt.bfloat16
AF = mybir.ActivationFunctionType
OP = mybir.AluOpType

B, T, DIM = 4, 4096, 1024
E = 1024
KC = 4  # conv taps
N_CORES = 8
T_LOC = T // 2      # 2048 tokens per core
TT = 512            # token tile
NTT = T_LOC // TT   # 4
NE = E // 128       # 8 channel chunks
NK = DIM // 128     # 8 contraction tiles


def _build_kernel(profile_mode=False):
    nc = bacc.Bacc("TRN2", target_bir_lowering=False, debug=False,
                   num_devices=1 if profile_mode else N_CORES)

    xT = nc.dram_tensor("xT", [DIM, T_LOC], F32, kind="ExternalInput")
    xa_halo = nc.dram_tensor("xa_halo", [E, KC - 1], F32, kind="ExternalInput")
    w_in_g = nc.dram_tensor("w_in_g", [DIM, E], F32, kind="ExternalInput")
    w_in_x = nc.dram_tensor("w_in_x", [DIM, E], F32, kind="ExternalInput")
    w_gates = nc.dram_tensor("w_gates", [E, 2 * E], F32, kind="ExternalInput")
    w_out = nc.dram_tensor("w_out", [E, DIM], F32, kind="ExternalInput")
    wc = nc.dram_tensor("wc", [E, KC], F32, kind="ExternalInput")
    b_conv = nc.dram_tensor("b_conv", [E, 1], F32, kind="ExternalInput")
    neg_c = nc.dram_tensor("neg_c", [E, 1], F32, kind="ExternalInput")
    b_f = nc.dram_tensor("b_f", [E, 1], F32, kind="ExternalInput")
    b_i = nc.dram_tensor("b_i", [E, 1], F32, kind="ExternalInput")
    mask_c = nc.dram_tensor("mask_c", [128, 1], F32, kind="ExternalInput")
    mask_u = nc.dram_tensor("mask_u", [128, 1], F32, kind="ExternalInput")
    out = nc.dram_tensor("out", [T_LOC, DIM], F32, kind="ExternalOutput")

    with tile.TileContext(nc) as tc, ExitStack() as ctx:
        _body(ctx, tc, nc, profile_mode=profile_mode,
              xT=xT, xa_halo=xa_halo, w_in_g=w_in_g,
              w_in_x=w_in_x, w_gates=w_gates, w_out=w_out, wc=wc,
              b_conv=b_conv, neg_c=neg_c, b_f=b_f, b_i=b_i,
              mask_c=mask_c, mask_u=mask_u, out=out)
    nc.compile()
    return nc


def _body(ctx, tc, nc, *, xT, xa_halo, w_in_g, w_in_x, w_gates, w_out, wc,
          b_conv, neg_c, b_f, b_i, mask_c, mask_u, out, profile_mode=False):
    consts = ctx.enter_context(tc.tile_pool(name="consts", bufs=1))
    ps1 = ctx.enter_context(tc.tile_pool(name="ps1", bufs=8, space="PSUM"))
    dram = ctx.enter_context(tc.tile_pool(name="dram", bufs=1, space="DRAM"))

    def load_chan_const(t_dram, n):
        t = consts.tile([128, NE, n], F32, tag=t_dram.name)
        nc.sync.dma_start(t[:], t_dram.ap().rearrange("(m p) n -> p m n", p=128))
        return t

    wc_sb = load_chan_const(wc, KC)
    b_conv_sb = load_chan_const(b_conv, 1)
    neg_c_sb = load_chan_const(neg_c, 1)
    b_f_sb = load_chan_const(b_f, 1)
    b_i_sb = load_chan_const(b_i, 1)
    mc_sb = consts.tile([128, 1], F32, tag="mc")
    nc.sync.dma_start(mc_sb[:], mask_c.ap()[:])
    mu_sb = consts.tile([128, 1], F32, tag="mu")
    nc.sync.dma_start(mu_sb[:], mask_u.ap()[:])
    zeros = consts.tile([128, TT], F32, tag="zeros")
    nc.vector.memset(zeros[:], 0.0)
    c_zero = consts.tile([128, 1], F32, tag="c_zero")
    nc.vector.memset(c_zero[:], 0.0)
    c_sqb = consts.tile([128, 1], F32, tag="c_sqb")
    nc.vector.memset(c_sqb[:], 1.000001)
    hcarry = consts.tile([128, NE], F32, tag="hcarry")
    carry = consts.tile([128, NE], F32, tag="carry")

    h_dram = dram.tile([NE, NTT, 128, TT], F32, tag="h_spill")
    p_dram = dram.tile([NE, NTT, 128, TT], F32, tag="p_spill")
    xc_dram = dram.tile([NTT, 128, NE, TT], F32R, tag="xc_spill")
    cc_in = dram.tile([E], F32, tag="cc_in")
    cc_out = dram.tile([E], F32, tag="cc_out")

    # w_gates loads up-front so it streams in while stage A computes
    wg_stack = ctx.enter_context(ExitStack())
    wg = wg_stack.enter_context(tc.tile_pool(name="w_gates", bufs=1, side="right"))
    wg_sb = wg.tile([128, NK, 2 * E], F32R)
    wg_src = w_gates.ap().rearrange("(k p) f -> p k f", p=128)

    # ============ Stage A: xa proj + causal conv -> xc (spilled) =======
    with ExitStack() as sa:
        wx = sa.enter_context(tc.tile_pool(name="w_in_x", bufs=1, side="right"))
        wx_sb = wx.tile([128, NK, E], F32R)
        wx_src = w_in_x.ap().rearrange("(k p) e -> p k e", p=128)
        xc_pool = sa.enter_context(tc.tile_pool(name="xc", bufs=2))
        xs_pool = sa.enter_context(tc.tile_pool(name="xstream", bufs=16))
        xa_pool = sa.enter_context(tc.tile_pool(name="xa", bufs=2))
        c3_pool = sa.enter_context(tc.tile_pool(name="c3", bufs=2))

        c3prev = c3_pool.tile([128, NE, KC - 1], F32, tag="c3")
        nc.sync.dma_start(c3prev[:],
                          xa_halo.ap().rearrange("(m p) n -> p m n", p=128))
        for tt in range(NTT):
            xtt = []
            for k in range(NK):
                if tt == 0:
                    nc.sync.dma_start(wx_sb[:, k], wx_src[:, k].bitcast(F32R))
                t = xs_pool.tile([128, TT], F32R, tag="xstream")
                nc.sync.dma_start(
                    t[:], xT.ap()[k * 128:(k + 1) * 128,
                                  tt * TT:(tt + 1) * TT].bitcast(F32R))
                xtt.append(t)
            for k in range(2 * tt, 2 * tt + 2):
                nc.sync.dma_start(wg_sb[:, k], wg_src[:, k].bitcast(F32R))
            xat = xa_pool.tile([128, NE, TT], F32, tag="xa")
            c3t = c3_pool.tile([128, NE, KC - 1], F32, tag="c3")
            for m in range(NE):
                pt = ps1.tile([128, TT], F32, tag="ps")
                for k in range(NK):
                    nc.tensor.matmul(pt[:], wx_sb[:, k, m * 128:(m + 1) * 128],
                                     xtt[k][:], start=(k == 0), stop=(k == NK - 1))
                nc.scalar.copy(xat[:, m], pt[:])
                nc.vector.tensor_copy(c3t[:, m], pt[:, TT - KC + 1:TT])
            xct = xc_pool.tile([128, NE, TT], F32R, tag="xc")
            for m in range(NE):
                # tap 0 (+bias), head from carry then main
                nc.vector.tensor_scalar(
                    xct[:, m, 0:KC - 1], c3prev[:, m], wc_sb[:, m, 0:1],
                    b_conv_sb[:, m, 0:1], op0=OP.mult, op1=OP.add)
                nc.vector.tensor_scalar(
                    xct[:, m, KC - 1:TT], xat[:, m, 0:TT - KC + 1],
                    wc_sb[:, m, 0:1], b_conv_sb[:, m, 0:1],
                    op0=OP.mult, op1=OP.add)
                for j in range(1, KC):
                    hw = KC - 1 - j  # head width
                    if hw > 0:
                        nc.vector.scalar_tensor_tensor(
                            xct[:, m, 0:hw], c3prev[:, m, j:KC - 1],
                            wc_sb[:, m, j:j + 1], xct[:, m, 0:hw].bitcast(F32),
                            op0=OP.mult, op1=OP.add)
                    nc.vector.scalar_tensor_tensor(
                        xct[:, m, hw:TT], xat[:, m, 0:TT - hw],
                        wc_sb[:, m, j:j + 1], xct[:, m, hw:TT].bitcast(F32),
                        op0=OP.mult, op1=OP.add)
            nc.sync.dma_start(xc_dram[tt], xct[:])
            c3prev = c3t

    # ============ Stage B: gates + elementwise + scans =================
    with ExitStack() as sb:
        # gate-projection weights load during stage B (used in stage D)
        wgt = sb.enter_context(tc.tile_pool(name="w_in_g", bufs=1))
        wg_in_sb = wgt.tile([128, NK, E], F32R)
        wgi_src = w_in_g.ap().rearrange("(k p) e -> p k e", p=128)
        dpre = sb.enter_context(tc.tile_pool(name="dpre", bufs=1))
        dpre_x = dpre.tile([128, 4, TT], F32R)
        sbw = sb.enter_context(ExitStack())
        xcs_pool = sbw.enter_context(tc.tile_pool(name="xcs", bufs=2))
        work = sbw.enter_context(tc.tile_pool(name="work", bufs=3))
        apool = sbw.enter_context(tc.tile_pool(name="apool", bufs=5))
        upool = sbw.enter_context(tc.tile_pool(name="upool", bufs=3))
        hp = sbw.enter_context(tc.tile_pool(name="hp", bufs=3))
        pp = sbw.enter_context(tc.tile_pool(name="pp", bufs=3))
        lc = sbw.enter_context(tc.tile_pool(name="lc", bufs=1))
        hl = {m: lc.tile([128, 1], F32, tag=f"hl{m}", name=f"hl{m}") for m in range(NE)}
        pl = {m: lc.tile([128, 1], F32, tag=f"pl{m}", name=f"pl{m}") for m in range(NE)}

        for tt in range(NTT):
            xct = xcs_pool.tile([128, NE, TT], F32R, tag="xcs")
            nc.sync.dma_start(xct[:], xc_dram[tt])
            for g4 in range(2):
                ms = range(g4 * 4, g4 * 4 + 4)
                pfs, pis, sfs, sis, alphas, us = {}, {}, {}, {}, {}, {}
                for m in ms:
                    pf = ps1.tile([128, TT], F32, tag="ps")
                    for k in range(NK):
                        nc.tensor.matmul(pf[:], wg_sb[:, k, m * 128:(m + 1) * 128],
                                         xct[:, k], start=(k == 0), stop=(k == NK - 1))
                    pfs[m] = pf
                    pi = ps1.tile([128, TT], F32, tag="ps")
                    for k in range(NK):
                        nc.tensor.matmul(pi[:], wg_sb[:, k, E + m * 128:E + (m + 1) * 128],
                                         xct[:, k], start=(k == 0), stop=(k == NK - 1))
                    pis[m] = pi
                for m in ms:
                    sf = work.tile([128, TT], F32, tag="sf")
                    nc.scalar.activation(sf[:], pfs[m][:], AF.Sigmoid,
                                         bias=b_f_sb[:, m, 0:1])
                    sfs[m] = sf
                    si = work.tile([128, TT], F32, tag="si")
                    nc.scalar.activation(si[:], pis[m][:], AF.Sigmoid,
                                         bias=b_i_sb[:, m, 0:1])
                    sis[m] = si
                for m in ms:
                    alpha = apool.tile([128, TT], F32, tag="alpha")
                    nc.scalar.activation(alpha[:], sfs[m][:], AF.Exp,
                                         scale=neg_c_sb[:, m, 0:1])
                    alphas[m] = alpha
                for m in ms:
                    asq = work.tile([128, TT], F32, tag="asq")
                    nc.vector.tensor_mul(asq[:], alphas[m][:], alphas[m][:])
                    sfs[m] = asq
                for m in ms:
                    nc.scalar.activation(sfs[m][:], sfs[m][:], AF.Sqrt,
                                         bias=c_sqb[:], scale=-1.0)
                for m in ms:
                    bs = work.tile([128, TT], F32, tag="bs")
                    nc.vector.tensor_mul(bs[:], sfs[m][:], sis[m][:])
                    u = upool.tile([128, TT], F32, tag="u")
                    nc.vector.tensor_mul(u[:], bs[:], xct[:, m].bitcast(F32))
                    us[m] = u
                for m in ms:
                    ht = hp.tile([128, TT], F32, tag="h")
                    nc.vector.tensor_tensor_scan(
                        ht[:], alphas[m][:], us[m][:],
                        0.0 if tt == 0 else hl[m][:],
                        op0=OP.mult, op1=OP.add)
                    nc.vector.tensor_copy(hl[m][:], ht[:, TT - 1:TT])
                    pt = pp.tile([128, TT], F32, tag="p")
                    nc.vector.tensor_tensor_scan(
                        pt[:], alphas[m][:], zeros[:],
                        1.0 if tt == 0 else pl[m][:],
                        op0=OP.mult, op1=OP.add)
                    nc.vector.tensor_copy(pl[m][:], pt[:, TT - 1:TT])
                    nc.sync.dma_start(h_dram[m, tt], ht[:])
                    nc.sync.dma_start(p_dram[m, tt], pt[:])
            for k in range(2 * tt, 2 * tt + 2):
                nc.sync.dma_start(wg_in_sb[:, k], wgi_src[:, k].bitcast(F32R))
            if tt == 2:
                for k in range(4):
                    nc.sync.dma_start(
                        dpre_x[:, k],
                        xT.ap()[k * 128:(k + 1) * 128, 0:TT].bitcast(F32R))
        for m in range(NE):
            nc.scalar.copy(hcarry[:, m:m + 1], hl[m][:])
        sbw.close()
        wg_stack.close()

        # ============ Stage C: pairwise carry exchange =================
        contrib = consts.tile([128, NE], F32, tag="contrib")
        nc.vector.tensor_scalar(contrib[:], hcarry[:], mc_sb[:, 0:1], None,
                                op0=OP.mult)
        nc.sync.dma_start(cc_in[:].rearrange("(j p) -> p j", p=128), contrib[:])
        if profile_mode:
            nc.sync.dma_start(cc_out[:], cc_in[:])
        else:
            nc.gpsimd.collective_compute(
                "AllReduce", OP.add,
                replica_groups=[[0, 1], [2, 3], [4, 5], [6, 7]],
                ins=[cc_in[:].opt()], outs=[cc_out[:].opt()])
        craw = consts.tile([128, NE], F32, tag="craw")
        nc.sync.dma_start(craw[:], cc_out[:].rearrange("(j p) -> p j", p=128))
        nc.vector.tensor_scalar(carry[:], craw[:], mu_sb[:, 0:1], None,
                                op0=OP.mult)

        # ============ Stage D: gate proj + correction + out proj =======
        with ExitStack() as sd:
            xs_pool = sd.enter_context(tc.tile_pool(name="xstream2", bufs=10))
            wo = sd.enter_context(tc.tile_pool(name="w_out", bufs=1))
            wo_sb = wo.tile([128, NK, DIM], F32R)
            wo_src = w_out.ap().rearrange("(k p) c -> p k c", p=128)
            gpool = sd.enter_context(tc.tile_pool(name="g", bufs=10))
            hs_pool = sd.enter_context(tc.tile_pool(name="hs", bufs=3))
            ypool = sd.enter_context(tc.tile_pool(name="y", bufs=12))
            opool = sd.enter_context(tc.tile_pool(name="osb", bufs=3))
            for tt in range(NTT):
                xtt = []
                for k in range(NK):
                    if tt == 0 and k < 4:
                        xtt.append(dpre_x[:, k])
                        continue
                    t = xs_pool.tile([128, TT], F32R, tag="xstream2")
                    nc.sync.dma_start(
                        t[:], xT.ap()[k * 128:(k + 1) * 128,
                                      tt * TT:(tt + 1) * TT].bitcast(F32R))
                    xtt.append(t)
                ys = []
                for m in range(NE):
                    pg = ps1.tile([128, TT], F32, tag="ps")
                    for k in range(NK):
                        nc.tensor.matmul(pg[:], wg_in_sb[:, k, m * 128:(m + 1) * 128],
                                         xtt[k][:], start=(k == 0), stop=(k == NK - 1))
                    g = gpool.tile([128, TT], F32, tag="g")
                    nc.scalar.activation(g[:], pg[:], AF.Gelu, bias=c_zero[:])
                    ht = hs_pool.tile([128, TT], F32, tag="hs")
                    nc.sync.dma_start(ht[:], h_dram[m, tt])
                    pt = hs_pool.tile([128, TT], F32, tag="pst")
                    nc.sync.dma_start(pt[:], p_dram[m, tt])
                    htrue = hs_pool.tile([128, TT], F32, tag="htrue")
                    nc.vector.scalar_tensor_tensor(
                        htrue[:], pt[:], carry[:, m:m + 1], ht[:],
                        op0=OP.mult, op1=OP.add)
                    y = ypool.tile([128, TT], F32R, tag="y")
                    nc.vector.tensor_mul(y[:], g[:], htrue[:])
                    ys.append(y)
                    if tt == 0:
                        nc.sync.dma_start(wo_sb[:, m], wo_src[:, m].bitcast(F32R))
                for q in range(TT // 128):
                    po0 = ps1.tile([128, 512], F32, tag="ps")
                    po1 = ps1.tile([128, 512], F32, tag="ps")
                    pos = [po0, po1]
                    for k in range(NE):
                        for n in range(DIM // 512):
                            nc.tensor.matmul(
                                pos[n][:],
                                ys[k][:, q * 128:(q + 1) * 128],
                                wo_sb[:, k, n * 512:(n + 1) * 512],
                                start=(k == 0), stop=(k == NE - 1))
                    osb = opool.tile([128, DIM], F32, tag="osb")
                    for n in range(2):
                        nc.scalar.copy(osb[:, n * 512:(n + 1) * 512], pos[n][:])
                    nc.sync.dma_start(
                        out.ap()[tt * TT + q * 128:tt * TT + (q + 1) * 128, :],
                        osb[:])


_NC_CACHE = {}


def _get_nc():
    if "nc" not in _NC_CACHE:
        _NC_CACHE["nc"] = _build_kernel()
    return _NC_CACHE["nc"]


def _softplus(x):
    return np.logaddexp(0.0, x)


def kernel(x, w_in, w_conv, b_conv, w_gates, b_gates, forget_base, w_out,
           _want_trace=False):
    x = np.asarray(x, dtype=np.float32)
    w_in = np.asarray(w_in, dtype=np.float32)
    w_conv = np.asarray(w_conv, dtype=np.float32)
    b_conv = np.asarray(b_conv, dtype=np.float32)
    w_gates = np.asarray(w_gates, dtype=np.float32)
    b_gates = np.asarray(b_gates, dtype=np.float32)
    forget_base = np.asarray(forget_base, dtype=np.float32)
    w_out = np.asarray(w_out, dtype=np.float32)

    nc = _get_nc()

    w_in_g = np.ascontiguousarray(w_in[:E].T)          # [DIM, E]
    w_in_x = np.ascontiguousarray(w_in[E:].T)          # [DIM, E]
    w_gates_T = np.ascontiguousarray(w_gates.T)        # [E, 2E]
    w_out_T = np.ascontiguousarray(w_out.T)            # [E, DIM]
    wc_r = np.ascontiguousarray(w_conv.reshape(E, KC))
    neg_c = (-8.0 * _softplus(forget_base.astype(np.float64))).astype(
        np.float32)[:, None]
    b_f = b_gates[:E, None].copy()
    b_i = b_gates[E:, None].copy()

    common = {
        "w_in_g": w_in_g, "w_in_x": w_in_x, "w_gates": w_gates_T,
        "w_out": w_out_T, "wc": wc_r, "b_conv": b_conv[:, None].copy(),
        "neg_c": neg_c, "b_f": b_f, "b_i": b_i,
    }
    in_maps = []
    for k in range(N_CORES):
        b, half = k // 2, k % 2
        t0 = half * T_LOC
        xT_loc = np.ascontiguousarray(x[b, t0:t0 + T_LOC, :].T)
        if half == 1:
            # xa for the 3 tokens before this chunk (for the causal conv)
            xa_halo = (x[b, t0 - (KC - 1):t0, :] @ w_in[E:].T).T
            xa_halo = np.ascontiguousarray(xa_halo)
        else:
            xa_halo = np.zeros((E, KC - 1), dtype=np.float32)
        mc = np.full((128, 1), 1.0 if half == 0 else 0.0, dtype=np.float32)
        mu = np.full((128, 1), 0.0 if half == 0 else 1.0, dtype=np.float32)
        in_maps.append({**common, "xT": xT_loc, "xa_halo": xa_halo,
                        "mask_c": mc, "mask_u": mu})

    res = run_bass_kernel_spmd(nc, in_maps, core_ids=list(range(N_CORES)),
                               trace=_want_trace)
    out_full = np.empty((B, T, DIM), dtype=np.float32)
    for k in range(N_CORES):
        b, half = k // 2, k % 2
        out_full[b, half * T_LOC:(half + 1) * T_LOC, :] = res.results[k]["out"]
    if _want_trace:
        return out_full, res
    return out_full



# revision 4
# speedup vs baseline: 1.0592x; 1.0592x over previous
"""Hawk (RG-LRU) block kernel for Trainium2, SPMD over 8 NeuronCores.

Sharding: tokens. Core k handles batch b=k//2, half h=k%2 (2048 tokens).
Weights replicated, host-transposed to matmul-ready layouts, cast bf16
(full PE rate, half the HBM traffic of f32).

Two fused passes over 4 token tiles of 512:
  pass 1: in-proj (xa) -> causal conv (DVE, bf16 2x) -> gates matmul ->
          sigmoid/exp/sqrt (batched per act table) -> u -> h-scan (DVE)
          + alpha-prefix-scan (Pool/GPSIMD); h,p spilled bf16.
  carry:  pairwise 4KB AllReduce moves the cross-half scan carry.
  pass 2: gate-proj + gelu, carry correction, out-proj; out stored bf16.
Out-proj runs one tile behind gate-proj so PE never waits on the
vector chain. DMA issue is spread over SP/Act/DVE/Pool queues to avoid
head-of-line blocking on one sequencer.
"""
import sys

sys.path.insert(0, "/opt/trn_rl_repo")

import numpy as np
import ml_dtypes
from contextlib import ExitStack

import concourse.bass as bass
import concourse.tile as tile
import concourse.bacc as bacc
from concourse import mybir
from concourse.bass_utils import run_bass_kernel_spmd

F32 = mybir.dt.float32
BF16 = mybir.dt.bfloat16
AF = mybir.ActivationFunctionType
OP = mybir.AluOpType

B, T, DIM = 4, 4096, 1024
E = 1024
KC = 4
N_CORES = 8
T_LOC = T // 2
TT = 512
NTT = T_LOC // TT   # 4
NE = E // 128       # 8
NK = DIM // 128     # 8


def _build_kernel(profile_mode=False):
    nc = bacc.Bacc("TRN2", target_bir_lowering=False, debug=False,
                   num_devices=1 if profile_mode else N_CORES)

    xT = nc.dram_tensor("xT", [DIM, T_LOC], BF16, kind="ExternalInput")
    xa_halo = nc.dram_tensor("xa_halo", [E, KC - 1], BF16, kind="ExternalInput")
    w_in_g = nc.dram_tensor("w_in_g", [DIM, E], BF16, kind="ExternalInput")
    w_in_x = nc.dram_tensor("w_in_x", [DIM, E], BF16, kind="ExternalInput")
    w_gates = nc.dram_tensor("w_gates", [E, 2 * E], BF16, kind="ExternalInput")
    w_out = nc.dram_tensor("w_out", [E, DIM], BF16, kind="ExternalInput")
    wc = nc.dram_tensor("wc", [E, KC], F32, kind="ExternalInput")
    b_conv = nc.dram_tensor("b_conv", [E, 1], F32, kind="ExternalInput")
    neg_c = nc.dram_tensor("neg_c", [E, 1], F32, kind="ExternalInput")
    neg_2c = nc.dram_tensor("neg_2c", [E, 1], F32, kind="ExternalInput")
    b_f = nc.dram_tensor("b_f", [E, 1], F32, kind="ExternalInput")
    b_i = nc.dram_tensor("b_i", [E, 1], F32, kind="ExternalInput")
    mask_c = nc.dram_tensor("mask_c", [128, 1], F32, kind="ExternalInput")
    mask_u = nc.dram_tensor("mask_u", [128, 1], F32, kind="ExternalInput")
    out = nc.dram_tensor("out", [T_LOC, DIM], BF16, kind="ExternalOutput")

    with tile.TileContext(nc) as tc, ExitStack() as ctx:
        _body(ctx, tc, nc, profile_mode=profile_mode,
              xT=xT, xa_halo=xa_halo, w_in_g=w_in_g, w_in_x=w_in_x,
              w_gates=w_gates, w_out=w_out, wc=wc, b_conv=b_conv,
              neg_c=neg_c, neg_2c=neg_2c, b_f=b_f, b_i=b_i,
              mask_c=mask_c, mask_u=mask_u, out=out)
    nc.compile()
    return nc


def _body(ctx, tc, nc, *, xT, xa_halo, w_in_g, w_in_x, w_gates, w_out, wc,
          b_conv, neg_c, neg_2c, b_f, b_i, mask_c, mask_u, out,
          profile_mode=False):
    consts = ctx.enter_context(tc.tile_pool(name="consts", bufs=1))
    ps = ctx.enter_context(tc.tile_pool(name="ps", bufs=8, space="PSUM"))
    dram = ctx.enter_context(tc.tile_pool(name="dram", bufs=1, space="DRAM"))
    wpool = ctx.enter_context(tc.tile_pool(name="weights", bufs=1, side="right"))
    xs = ctx.enter_context(tc.tile_pool(name="xs", bufs=3))
    xap = ctx.enter_context(tc.tile_pool(name="xap", bufs=2))
    xcp = ctx.enter_context(tc.tile_pool(name="xcp", bufs=2))
    sfp = ctx.enter_context(tc.tile_pool(name="sfp", bufs=3))
    sip = ctx.enter_context(tc.tile_pool(name="sip", bufs=3))
    alp = ctx.enter_context(tc.tile_pool(name="alp", bufs=5))
    a2p = ctx.enter_context(tc.tile_pool(name="a2p", bufs=3))
    bep = ctx.enter_context(tc.tile_pool(name="bep", bufs=3))
    bsp = ctx.enter_context(tc.tile_pool(name="bsp", bufs=2))
    up = ctx.enter_context(tc.tile_pool(name="up", bufs=3))
    htp = ctx.enter_context(tc.tile_pool(name="htp", bufs=4))
    ptp = ctx.enter_context(tc.tile_pool(name="ptp", bufs=4))
    hvp = ctx.enter_context(tc.tile_pool(name="hvp", bufs=3))
    osbp = ctx.enter_context(tc.tile_pool(name="osbp", bufs=3))

    # --- constants (issued on Act queue; SP stays free for weights/x) ---
    def chan_const(t_dram, n):
        t = consts.tile([128, NE, n], F32, tag=t_dram.name, name=t_dram.name)
        nc.scalar.dma_start(t[:], t_dram.ap().rearrange("(m p) n -> p m n", p=128))
        return t

    wc_sb = chan_const(wc, KC)
    bc_sb = chan_const(b_conv, 1)
    ncc_sb = chan_const(neg_c, 1)
    nc2_sb = chan_const(neg_2c, 1)
    bf_sb = chan_const(b_f, 1)
    bi_sb = chan_const(b_i, 1)
    mc_sb = consts.tile([128, 1], F32, tag="mc")
    nc.scalar.dma_start(mc_sb[:], mask_c.ap()[:])
    mu_sb = consts.tile([128, 1], F32, tag="mu")
    nc.scalar.dma_start(mu_sb[:], mask_u.ap()[:])
    zeros = consts.tile([128, TT], F32, tag="zeros")
    nc.vector.memset(zeros[:], 0.0)
    c_sqb = consts.tile([128, 1], F32, tag="c_sqb")
    nc.vector.memset(c_sqb[:], 1.000001)
    hc = consts.tile([128, NE], F32, tag="hc")      # h-scan carries per m
    plc = consts.tile([128, NE], F32, tag="plc")    # p-scan carries per m
    contrib = consts.tile([128, NE], F32, tag="contrib")
    craw = consts.tile([128, NE], F32, tag="craw")
    carry = consts.tile([128, NE], F32, tag="carry")

    h_dram = dram.tile([NE, NTT, 128, TT], BF16, tag="h_spill")
    p_dram = dram.tile([NE, NTT, 128, TT], BF16, tag="p_spill")
    cc_in = dram.tile([E], F32, tag="cc_in")
    cc_out = dram.tile([E], F32, tag="cc_out")

    # --- weights (persistent, bf16) ---
    wx_sb = wpool.tile([128, NK, E], BF16)
    wg_sb = wpool.tile([128, NK, 2 * E], BF16)
    wgi_sb = wpool.tile([128, NK, E], BF16)
    wo_sb = wpool.tile([128, NK, DIM], BF16)
    wx_src = w_in_x.ap().rearrange("(k p) e -> p k e", p=128)
    wg_src = w_gates.ap().rearrange("(k p) f -> p k f", p=128)
    wgi_src = w_in_g.ap().rearrange("(k p) e -> p k e", p=128)
    wo_src = w_out.ap().rearrange("(k p) c -> p k c", p=128)

    def load_x_tile(tt):
        t = xs.tile([128, NK, TT], BF16, tag="xstream", name="xt")
        for k in range(NK):
            nc.sync.dma_start(
                t[:, k], xT.ap()[k * 128:(k + 1) * 128, tt * TT:(tt + 1) * TT])
        return t

    # ================= pass 1: xa proj + conv + gates + scans ==========
    prev_xa = None
    xt_p2 = None
    for tt in range(NTT):
        if tt == 0:
            xt = xs.tile([128, NK, TT], BF16, tag="xstream", name="xt")
            for k in range(NK):
                nc.sync.dma_start(wx_sb[:, k], wx_src[:, k])
                nc.sync.dma_start(
                    xt[:, k],
                    xT.ap()[k * 128:(k + 1) * 128, 0:TT])
            for k in range(NK):
                nc.sync.dma_start(wg_sb[:, k], wg_src[:, k])
        else:
            xt = load_x_tile(tt)

        # xa projection
        xa = xap.tile([128, NE, TT + KC - 1], BF16, tag="xa", name="xa")
        if tt == 0:
            nc.scalar.dma_start(
                xa[:, :, 0:KC - 1],
                xa_halo.ap().rearrange("(m p) n -> p m n", p=128))
        else:
            nc.vector.tensor_copy(xa[:, :, 0:KC - 1],
                                  prev_xa[:, :, TT:TT + KC - 1])
        for m in range(NE):
            pa = ps.tile([128, TT], F32, tag="ps", name="pa")
            for k in range(NK):
                nc.tensor.matmul(pa[:], wx_sb[:, k, m * 128:(m + 1) * 128],
                                 xt[:, k], start=(k == 0), stop=(k == NK - 1))
            nc.scalar.copy(xa[:, m, KC - 1:TT + KC - 1], pa[:])

        # causal depthwise conv (bf16 on DVE)
        xc = xcp.tile([128, NE, TT], BF16, tag="xc", name="xc")
        for m in range(NE):
            nc.vector.tensor_scalar(
                xc[:, m], xa[:, m, 0:TT], wc_sb[:, m, 0:1], bc_sb[:, m, 0:1],
                op0=OP.mult, op1=OP.add)
            for j in range(1, KC):
                nc.vector.scalar_tensor_tensor(
                    xc[:, m], xa[:, m, j:j + TT], wc_sb[:, m, j:j + 1],
                    xc[:, m], op0=OP.mult, op1=OP.add)

        # gates + nonlinearities + scans, in 2 groups of 4 m (8 PSUM banks)
        for g in range(2):
            ms = range(g * 4, g * 4 + 4)
            pfs, pis, sfs, sis, als, bes = {}, {}, {}, {}, {}, {}
            for m in ms:
                pf = ps.tile([128, TT], F32, tag="ps", name="pf")
                for k in range(NK):
                    nc.tensor.matmul(pf[:], wg_sb[:, k, m * 128:(m + 1) * 128],
                                     xc[:, k], start=(k == 0), stop=(k == NK - 1))
                pfs[m] = pf
                pi = ps.tile([128, TT], F32, tag="ps", name="pi")
                for k in range(NK):
                    nc.tensor.matmul(pi[:], wg_sb[:, k, E + m * 128:E + (m + 1) * 128],
                                     xc[:, k], start=(k == 0), stop=(k == NK - 1))
                pis[m] = pi
            for m in ms:  # sigmoid table
                sf = sfp.tile([128, TT], F32, tag="sf", name="sf")
                nc.scalar.activation(sf[:], pfs[m][:], AF.Sigmoid,
                                     bias=bf_sb[:, m, 0:1])
                sfs[m] = sf
                si = sip.tile([128, TT], F32, tag="si", name="si")
                nc.scalar.activation(si[:], pis[m][:], AF.Sigmoid,
                                     bias=bi_sb[:, m, 0:1])
                sis[m] = si
            for m in ms:  # exp table
                al = alp.tile([128, TT], F32, tag="al", name="al")
                nc.scalar.activation(al[:], sfs[m][:], AF.Exp,
                                     scale=ncc_sb[:, m, 0:1])
                als[m] = al
                a2 = a2p.tile([128, TT], F32, tag="a2", name="a2")
                nc.scalar.activation(a2[:], sfs[m][:], AF.Exp,
                                     scale=nc2_sb[:, m, 0:1])
                bes[m] = a2
            for m in ms:  # sqrt table: beta = sqrt(1.000001 - alpha^2)
                be = bep.tile([128, TT], F32, tag="be", name="be")
                nc.scalar.activation(be[:], bes[m][:], AF.Sqrt,
                                     bias=c_sqb[:], scale=-1.0)
                bes[m] = be
            for m in ms:
                bs = bsp.tile([128, TT], F32, tag="bs", name="bs")
                nc.vector.tensor_mul(bs[:], bes[m][:], sis[m][:])
                u = up.tile([128, TT], F32, tag="u", name="u")
                nc.vector.tensor_mul(u[:], bs[:], xc[:, m])
                ht = htp.tile([128, TT], BF16, tag="ht", name="ht")
                nc.vector.tensor_tensor_scan(
                    ht[:], als[m][:], u[:],
                    0.0 if tt == 0 else hc[:, m:m + 1],
                    op0=OP.mult, op1=OP.add)
                nc.vector.tensor_copy(hc[:, m:m + 1], ht[:, TT - 1:TT])
                nc.scalar.dma_start(h_dram[m, tt], ht[:])
                pt = ptp.tile([128, TT], BF16, tag="pt", name="pt")
                nc.vector.tensor_tensor_scan(
                    pt[:], als[m][:], zeros[:],
                    1.0 if tt == 0 else plc[:, m:m + 1],
                    op0=OP.mult, op1=OP.add)
                nc.vector.tensor_copy(plc[:, m:m + 1], pt[:, TT - 1:TT])
                nc.gpsimd.dma_start(p_dram[m, tt], pt[:])

        # prefetch pass-2 weights while pass 1 computes
        for k in (2 * tt, 2 * tt + 1):
            nc.sync.dma_start(wgi_sb[:, k], wgi_src[:, k])
            nc.sync.dma_start(wo_sb[:, k], wo_src[:, k])
        prev_xa = xa
        if tt == NTT - 1:
            xt_p2 = load_x_tile(0)  # prefetch pass-2 first tile

    # ================= carry exchange (pairwise AllReduce, 4KB) ========
    nc.vector.tensor_scalar(contrib[:], hc[:], mc_sb[:, 0:1], None,
                            op0=OP.mult)
    nc.sync.dma_start(cc_in[:].rearrange("(j p) -> p j", p=128), contrib[:])
    if profile_mode:
        nc.sync.dma_start(cc_out[:], cc_in[:])
    else:
        nc.gpsimd.collective_compute(
            "AllReduce", OP.add,
            replica_groups=[[0, 1], [2, 3], [4, 5], [6, 7]],
            ins=[cc_in[:].opt()], outs=[cc_out[:].opt()])
    nc.sync.dma_start(craw[:], cc_out[:].rearrange("(j p) -> p j", p=128))
    nc.vector.tensor_scalar(carry[:], craw[:], mu_sb[:, 0:1], None,
                            op0=OP.mult)

    # ================= pass 2: gate proj + correction + out proj =======
    ys = {}
    for step in range(NTT + 1):
        if step < NTT:
            tt = step
            xt = xt_p2 if tt == 0 else load_x_tile(tt)
            y = xap.tile([128, NE, TT + KC - 1], BF16, tag="xa", name="y")
            gg = xcp.tile([128, NE, TT], BF16, tag="xc", name="gg")
            for m in range(NE):
                pg = ps.tile([128, TT], F32, tag="ps", name="pg")
                for k in range(NK):
                    nc.tensor.matmul(pg[:], wgi_sb[:, k, m * 128:(m + 1) * 128],
                                     xt[:, k], start=(k == 0), stop=(k == NK - 1))
                nc.scalar.activation(gg[:, m], pg[:], AF.Gelu)
                ht = htp.tile([128, TT], BF16, tag="ht", name="ht2")
                nc.sync.dma_start(ht[:], h_dram[m, tt])
                pt = ptp.tile([128, TT], BF16, tag="pt", name="pt2")
                nc.sync.dma_start(pt[:], p_dram[m, tt])
                hv = hvp.tile([128, TT], F32, tag="hv", name="hv")
                nc.vector.scalar_tensor_tensor(
                    hv[:], pt[:], carry[:, m:m + 1], ht[:],
                    op0=OP.mult, op1=OP.add)
                nc.vector.tensor_mul(y[:, m, 0:TT], gg[:, m], hv[:])
            ys[tt] = y
        if step >= 1:
            tt = step - 1
            y = ys.pop(tt)
            for q in range(TT // 128):
                pos = [ps.tile([128, 512], F32, tag="ps", name="po")
                       for _ in range(2)]
                for k in range(NE):
                    for n in range(2):
                        nc.tensor.matmul(
                            pos[n][:], y[:, k, q * 128:(q + 1) * 128],
                            wo_sb[:, k, n * 512:(n + 1) * 512],
                            start=(k == 0), stop=(k == NE - 1))
                osb = osbp.tile([128, DIM], BF16, tag="osb", name="osb")
                for n in range(2):
                    nc.scalar.copy(osb[:, n * 512:(n + 1) * 512], pos[n][:])
                nc.sync.dma_start(
                    out.ap()[tt * TT + q * 128:tt * TT + (q + 1) * 128, :],
                    osb[:])


_NC_CACHE = {}


def _get_nc():
    if "nc" not in _NC_CACHE:
        _NC_CACHE["nc"] = _build_kernel()
    return _NC_CACHE["nc"]


def _softplus(x):
    return np.logaddexp(0.0, x)


def kernel(x, w_in, w_conv, b_conv, w_gates, b_gates, forget_base, w_out,
           _want_trace=False):
    BF = ml_dtypes.bfloat16
    x = np.asarray(x, dtype=np.float32)
    w_in = np.asarray(w_in, dtype=np.float32)
    w_conv = np.asarray(w_conv, dtype=np.float32)
    b_conv = np.asarray(b_conv, dtype=np.float32)
    w_gates = np.asarray(w_gates, dtype=np.float32)
    b_gates = np.asarray(b_gates, dtype=np.float32)
    forget_base = np.asarray(forget_base, dtype=np.float32)
    w_out = np.asarray(w_out, dtype=np.float32)

    nc = _get_nc()

    w_in_g = np.ascontiguousarray(w_in[:E].T).astype(BF)     # [DIM, E]
    w_in_x = np.ascontiguousarray(w_in[E:].T).astype(BF)     # [DIM, E]
    w_gates_T = np.ascontiguousarray(w_gates.T).astype(BF)   # [E, 2E]
    w_out_T = np.ascontiguousarray(w_out.T).astype(BF)       # [E, DIM]
    wc_r = np.ascontiguousarray(w_conv.reshape(E, KC))
    neg_c = (-8.0 * _softplus(forget_base.astype(np.float64))).astype(
        np.float32)[:, None]
    b_f = b_gates[:E, None].copy()
    b_i = b_gates[E:, None].copy()

    common = {
        "w_in_g": w_in_g, "w_in_x": w_in_x, "w_gates": w_gates_T,
        "w_out": w_out_T, "wc": wc_r, "b_conv": b_conv[:, None].copy(),
        "neg_c": neg_c, "neg_2c": 2.0 * neg_c, "b_f": b_f, "b_i": b_i,
    }
    in_maps = []
    for k in range(N_CORES):
        b, half = k // 2, k % 2
        t0 = half * T_LOC
        xT_loc = np.ascontiguousarray(x[b, t0:t0 + T_LOC, :].T).astype(BF)
        if half == 1:
            xa_halo = (x[b, t0 - (KC - 1):t0, :] @ w_in[E:].T).T
            xa_halo = np.ascontiguousarray(xa_halo).astype(BF)
        else:
            xa_halo = np.zeros((E, KC - 1), dtype=BF)
        mc = np.full((128, 1), 1.0 if half == 0 else 0.0, dtype=np.float32)
        mu = np.full((128, 1), 0.0 if half == 0 else 1.0, dtype=np.float32)
        in_maps.append({**common, "xT": xT_loc, "xa_halo": xa_halo,
                        "mask_c": mc, "mask_u": mu})

    res = run_bass_kernel_spmd(nc, in_maps, core_ids=list(range(N_CORES)),
                               trace=_want_trace)
    out_full = np.empty((B, T, DIM), dtype=np.float32)
    for k in range(N_CORES):
        b, half = k // 2, k % 2
        out_full[b, half * T_LOC:(half + 1) * T_LOC, :] = \
            res.results[k]["out"].astype(np.float32)
    if _want_trace:
        return out_full, res
    return out_full


# revision 8
# speedup vs baseline: 1.1118x; 1.0496x over previous
"""Hawk (RG-LRU) block kernel for Trainium2, SPMD over 8 NeuronCores.

Sharding: tokens. Core k handles batch b=k//2, half h=k%2 (2048 tokens).
Weights replicated, host-transposed, bf16 (full PE rate, half the HBM
traffic). Two fused passes over 4 token tiles of 512:

  pass 1: in-proj -> causal conv (DVE, bf16) -> gates matmul ->
          tanh/exp/ln activation chain (sigmoid via tanh so tanh+exp
          share one act-func table; beta via ln+exp) -> u ->
          h-scan + alpha-prefix-scan (DVE, fp32 state, bf16 out);
          h,p spill bf16 via the idle GPSIMD DMA queue.
  carry:  pairwise 4KB AllReduce moves the cross-half scan carry.
  pass 2: gate-proj + gelu, carry correction, out-proj one tile behind
          so PE never waits on the vector chain; out stored bf16.

alpha^2 runs on the (otherwise idle) GPSIMD engine. DMAs are batched
(one per tile per stream) and spread over SP/Act/Pool queues to avoid
sequencer head-of-line blocking.
"""
import sys

sys.path.insert(0, "/opt/trn_rl_repo")

import numpy as np
import ml_dtypes
from contextlib import ExitStack

import concourse.bass as bass
import concourse.tile as tile
import concourse.bacc as bacc
from concourse import mybir
from concourse.bass_utils import run_bass_kernel_spmd

F32 = mybir.dt.float32
BF16 = mybir.dt.bfloat16
AF = mybir.ActivationFunctionType
OP = mybir.AluOpType

B, T, DIM = 4, 4096, 1024
E = 1024
KC = 4
N_CORES = 8
T_LOC = T // 2
TT = 512
NTT = T_LOC // TT   # 4
NE = E // 128       # 8
NK = DIM // 128     # 8

ALPHA2_ON_POOL = True


def _build_kernel(profile_mode=False):
    nc = bacc.Bacc("TRN2", target_bir_lowering=False, debug=False,
                   num_devices=1 if profile_mode else N_CORES)

    xT = nc.dram_tensor("xT", [DIM, T_LOC], BF16, kind="ExternalInput")
    xa_halo = nc.dram_tensor("xa_halo", [E, KC - 1], BF16, kind="ExternalInput")
    w_in_g = nc.dram_tensor("w_in_g", [DIM, E], BF16, kind="ExternalInput")
    w_in_x = nc.dram_tensor("w_in_x", [DIM, E], BF16, kind="ExternalInput")
    w_gates = nc.dram_tensor("w_gates", [E, 2 * E], BF16, kind="ExternalInput")
    w_out = nc.dram_tensor("w_out", [E, DIM], BF16, kind="ExternalInput")
    wc = nc.dram_tensor("wc", [E, KC], F32, kind="ExternalInput")
    b_conv = nc.dram_tensor("b_conv", [E, 1], F32, kind="ExternalInput")
    neg_ch = nc.dram_tensor("neg_ch", [E, 1], F32, kind="ExternalInput")
    b_fh = nc.dram_tensor("b_fh", [E, 1], F32, kind="ExternalInput")
    b_ih = nc.dram_tensor("b_ih", [E, 1], F32, kind="ExternalInput")
    mask_c = nc.dram_tensor("mask_c", [128, 1], F32, kind="ExternalInput")
    mask_u = nc.dram_tensor("mask_u", [128, 1], F32, kind="ExternalInput")
    out = nc.dram_tensor("out", [T_LOC, DIM], BF16, kind="ExternalOutput")

    with tile.TileContext(nc) as tc, ExitStack() as ctx:
        _body(ctx, tc, nc, profile_mode=profile_mode,
              xT=xT, xa_halo=xa_halo, w_in_g=w_in_g, w_in_x=w_in_x,
              w_gates=w_gates, w_out=w_out, wc=wc, b_conv=b_conv,
              neg_ch=neg_ch, b_fh=b_fh, b_ih=b_ih,
              mask_c=mask_c, mask_u=mask_u, out=out)
    nc.compile()
    return nc


def _body(ctx, tc, nc, *, xT, xa_halo, w_in_g, w_in_x, w_gates, w_out, wc,
          b_conv, neg_ch, b_fh, b_ih, mask_c, mask_u, out,
          profile_mode=False):
    consts = ctx.enter_context(tc.tile_pool(name="consts", bufs=1))
    ps = ctx.enter_context(tc.tile_pool(name="ps", bufs=8, space="PSUM"))
    dram = ctx.enter_context(tc.tile_pool(name="dram", bufs=1, space="DRAM"))
    wpool = ctx.enter_context(tc.tile_pool(name="weights", bufs=1, side="right"))
    xs = ctx.enter_context(tc.tile_pool(name="xs", bufs=2))
    xap = ctx.enter_context(tc.tile_pool(name="xap", bufs=2))
    xcp = ctx.enter_context(tc.tile_pool(name="xcp", bufs=2))
    sfp = ctx.enter_context(tc.tile_pool(name="sfp", bufs=3))
    sip = ctx.enter_context(tc.tile_pool(name="sip", bufs=4))
    alp = ctx.enter_context(tc.tile_pool(name="alp", bufs=6))
    a2p = ctx.enter_context(tc.tile_pool(name="a2p", bufs=4))
    bep = ctx.enter_context(tc.tile_pool(name="bep", bufs=4))
    bsp = ctx.enter_context(tc.tile_pool(name="bsp", bufs=2))
    up = ctx.enter_context(tc.tile_pool(name="up", bufs=3))
    spl = ctx.enter_context(tc.tile_pool(name="spl", bufs=2))
    hvp = ctx.enter_context(tc.tile_pool(name="hvp", bufs=2))
    osbp = ctx.enter_context(tc.tile_pool(name="osbp", bufs=2))

    # --- constants (Act queue keeps SP free for weights/x) ---
    def chan_const(t_dram, n):
        t = consts.tile([128, NE, n], F32, tag=t_dram.name, name=t_dram.name)
        nc.scalar.dma_start(t[:], t_dram.ap().rearrange("(m p) n -> p m n", p=128))
        return t

    wc_sb = chan_const(wc, KC)
    bc_sb = chan_const(b_conv, 1)
    nch_sb = chan_const(neg_ch, 1)
    bfh_sb = chan_const(b_fh, 1)
    bih_sb = chan_const(b_ih, 1)
    mc_sb = consts.tile([128, 1], F32, tag="mc")
    nc.scalar.dma_start(mc_sb[:], mask_c.ap()[:])
    mu_sb = consts.tile([128, 1], F32, tag="mu")
    nc.scalar.dma_start(mu_sb[:], mask_u.ap()[:])
    zeros = consts.tile([128, TT], F32, tag="zeros")
    nc.vector.memset(zeros[:], 0.0)
    c_zero = consts.tile([128, 1], F32, tag="c_zero")
    nc.vector.memset(c_zero[:], 0.0)
    c_qb = consts.tile([128, 1], F32, tag="c_qb")
    nc.vector.memset(c_qb[:], 0.25000025)
    hc = consts.tile([128, NE], F32, tag="hc")
    plc = consts.tile([128, NE], F32, tag="plc")
    contrib = consts.tile([128, NE], F32, tag="contrib")
    craw = consts.tile([128, NE], F32, tag="craw")
    carry = consts.tile([128, NE], F32, tag="carry")

    h_dram = dram.tile([NTT, 128, NE, TT], BF16, tag="h_spill")
    p_dram = dram.tile([NTT, 128, NE, TT], BF16, tag="p_spill")
    cc_in = dram.tile([E], F32, tag="cc_in")
    cc_out = dram.tile([E], F32, tag="cc_out")

    # --- weights (persistent bf16; w_out shares the w_in_x slot) ---
    wx_sb = wpool.tile([128, NK, E], BF16, tag="wxo", name="wx_sb")
    wg_sb = wpool.tile([128, NK, 2 * E], BF16, tag="wg", name="wg_sb")
    wgi_sb = wpool.tile([128, NK, E], BF16, tag="wgi", name="wgi_sb")
    wx_src = w_in_x.ap().rearrange("(k p) e -> p k e", p=128)
    wg_src = w_gates.ap().rearrange("(k p) f -> p k f", p=128)
    wgi_src = w_in_g.ap().rearrange("(k p) e -> p k e", p=128)
    wo_src = w_out.ap().rearrange("(k p) c -> p k c", p=128)
    xT_r = xT.ap().rearrange("(k p) t -> p k t", p=128)
    halo_r = xa_halo.ap().rearrange("(m p) n -> p m n", p=128)

    def load_x_tile(tt):
        t = xs.tile([128, NK, TT], BF16, tag="xstream", name="xt")
        nc.sync.dma_start(t[:], xT_r[:, :, tt * TT:(tt + 1) * TT])
        return t

    # ================= pass 1: xa proj + conv + gates + scans ==========
    prev_xa = None
    xt_p2 = None
    for tt in range(NTT):
        if tt == 0:
            xt = xs.tile([128, NK, TT], BF16, tag="xstream", name="xt")
            for k in range(NK):
                nc.sync.dma_start(wx_sb[:, k], wx_src[:, k])
                nc.sync.dma_start(xt[:, k], xT_r[:, k, 0:TT])
            for k in range(NK):
                nc.sync.dma_start(wg_sb[:, k], wg_src[:, k])
        else:
            xt = load_x_tile(tt)

        # xa projection + causal conv, per channel chunk
        xas, xcs = [], []
        for m in range(NE):
            pa = ps.tile([128, TT], F32, tag="ps", name="pa")
            for k in range(NK):
                nc.tensor.matmul(pa[:], wx_sb[:, k, m * 128:(m + 1) * 128],
                                 xt[:, k], start=(k == 0), stop=(k == NK - 1))
            xa = xap.tile([128, TT + KC - 1], BF16, tag=f"xa{m}", name="xa")
            nc.scalar.copy(xa[:, KC - 1:TT + KC - 1], pa[:])
            if tt == 0:
                nc.scalar.dma_start(xa[:, 0:KC - 1], halo_r[:, m])
            else:
                nc.vector.tensor_copy(xa[:, 0:KC - 1],
                                      prev_xa[m][:, TT:TT + KC - 1])
            xc = xcp.tile([128, TT], BF16, tag=f"xc{m}", name="xc")
            nc.vector.tensor_scalar(
                xc[:], xa[:, 0:TT], wc_sb[:, m, 0:1], bc_sb[:, m, 0:1],
                op0=OP.mult, op1=OP.add)
            for j in range(1, KC):
                nc.vector.scalar_tensor_tensor(
                    xc[:], xa[:, j:j + TT], wc_sb[:, m, j:j + 1],
                    xc[:], op0=OP.mult, op1=OP.add)
            xas.append(xa)
            xcs.append(xc)

        # gates matmuls + activations; tanh & exp share one act table
        sfs, sis, als, a2s, bes = {}, {}, {}, {}, {}
        for g in range(2):
            ms = range(g * 4, g * 4 + 4)
            pfs, pis = {}, {}
            for m in ms:
                pf = ps.tile([128, TT], F32, tag="ps", name="pf")
                for k in range(NK):
                    nc.tensor.matmul(pf[:], wg_sb[:, k, m * 128:(m + 1) * 128],
                                     xcs[k][:], start=(k == 0), stop=(k == NK - 1))
                pfs[m] = pf
                pi = ps.tile([128, TT], F32, tag="ps", name="pi")
                for k in range(NK):
                    nc.tensor.matmul(pi[:], wg_sb[:, k, E + m * 128:E + (m + 1) * 128],
                                     xcs[k][:], start=(k == 0), stop=(k == NK - 1))
                pis[m] = pi
            for m in ms:  # sigmoid(x) = 0.5*tanh(x/2) + 0.5, folded downstream
                sf = sfp.tile([128, TT], F32, tag="sf", name="sf")
                nc.scalar.activation(sf[:], pfs[m][:], AF.Tanh,
                                     scale=0.5, bias=bfh_sb[:, m, 0:1])
                sfs[m] = sf
                si = sip.tile([128, TT], F32, tag="si", name="si")
                nc.scalar.activation(si[:], pis[m][:], AF.Tanh,
                                     scale=0.5, bias=bih_sb[:, m, 0:1])
                sis[m] = si
            for m in ms:  # alpha = exp(-c*sigmoid(f)) = exp(-c/2*vf - c/2)
                al = alp.tile([128, TT], F32, tag="al", name="al")
                nc.scalar.activation(al[:], sfs[m][:], AF.Exp,
                                     scale=nch_sb[:, m, 0:1],
                                     bias=nch_sb[:, m, 0:1])
                als[m] = al
                a2 = a2p.tile([128, TT], F32, tag="a2", name="a2")
                if ALPHA2_ON_POOL:
                    nc.gpsimd.tensor_mul(a2[:], al[:], al[:])
                else:
                    nc.vector.tensor_mul(a2[:], al[:], al[:])
                a2s[m] = a2
        for m in range(NE):  # beta/2 = sqrt(0.25000025 - 0.25*alpha^2) via ln+exp
            lnb = bep.tile([128, TT], F32, tag="be", name="lnb")
            nc.scalar.activation(lnb[:], a2s[m][:], AF.Ln,
                                 scale=-0.25, bias=c_qb[:])
            nc.scalar.activation(lnb[:], lnb[:], AF.Exp, scale=0.5,
                                 bias=c_zero[:])
            bes[m] = lnb

        h_all = spl.tile([128, NE, TT], BF16, tag="hall", name="h_all")
        p_all = spl.tile([128, NE, TT], BF16, tag="pall", name="p_all")
        for m in range(NE):
            bs = bsp.tile([128, TT], F32, tag="bs", name="bs")
            nc.vector.scalar_tensor_tensor(bs[:], sis[m][:], 1.0, bes[m][:],
                                           op0=OP.add, op1=OP.mult)
            u = up.tile([128, TT], F32, tag="u", name="u")
            nc.vector.tensor_mul(u[:], bs[:], xcs[m][:])
            nc.vector.tensor_tensor_scan(
                h_all[:, m], als[m][:], u[:],
                0.0 if tt == 0 else hc[:, m:m + 1],
                op0=OP.mult, op1=OP.add)
            nc.vector.tensor_copy(hc[:, m:m + 1], h_all[:, m, TT - 1:TT])
            nc.vector.tensor_tensor_scan(
                p_all[:, m], als[m][:], zeros[:],
                1.0 if tt == 0 else plc[:, m:m + 1],
                op0=OP.mult, op1=OP.add)
            nc.vector.tensor_copy(plc[:, m:m + 1], p_all[:, m, TT - 1:TT])
        nc.gpsimd.dma_start(h_dram[tt], h_all[:])
        nc.gpsimd.dma_start(p_dram[tt], p_all[:])

        for k in (2 * tt, 2 * tt + 1):  # pass-2 gate weights prefetch
            nc.sync.dma_start(wgi_sb[:, k], wgi_src[:, k])
        prev_xa = xas
        if tt == NTT - 1:
            xt_p2 = load_x_tile(0)

    # w_out loads into the (now dead) w_in_x slot
    wo_sb = wpool.tile([128, NK, DIM], BF16, tag="wxo", name="wo_sb")
    for k in range(NK):
        nc.sync.dma_start(wo_sb[:, k], wo_src[:, k])

    # ================= carry exchange (pairwise AllReduce, 4KB) ========
    nc.vector.tensor_scalar(contrib[:], hc[:], mc_sb[:, 0:1], None,
                            op0=OP.mult)
    nc.sync.dma_start(cc_in[:].rearrange("(j p) -> p j", p=128), contrib[:])
    if profile_mode:
        nc.sync.dma_start(cc_out[:], cc_in[:])
    else:
        nc.gpsimd.collective_compute(
            "AllReduce", OP.add,
            replica_groups=[[0, 1], [2, 3], [4, 5], [6, 7]],
            ins=[cc_in[:].opt()], outs=[cc_out[:].opt()])
    nc.sync.dma_start(craw[:], cc_out[:].rearrange("(j p) -> p j", p=128))
    nc.vector.tensor_scalar(carry[:], craw[:], mu_sb[:, 0:1], None,
                            op0=OP.mult)

    # ================= pass 2: gate proj + correction + out proj =======
    ys = {}
    for step in range(NTT + 1):
        if step < NTT:
            tt = step
            xt = xt_p2 if tt == 0 else load_x_tile(tt)
            h2 = spl.tile([128, NE, TT], BF16, tag="hall", name="h2")
            nc.sync.dma_start(h2[:], h_dram[tt])
            p2 = spl.tile([128, NE, TT], BF16, tag="pall", name="p2")
            nc.sync.dma_start(p2[:], p_dram[tt])
            yt = []
            for m in range(NE):
                pg = ps.tile([128, TT], F32, tag="ps", name="pg")
                for k in range(NK):
                    nc.tensor.matmul(pg[:], wgi_sb[:, k, m * 128:(m + 1) * 128],
                                     xt[:, k], start=(k == 0), stop=(k == NK - 1))
                gg = xcp.tile([128, TT], BF16, tag=f"xc{m}", name="gg")
                nc.scalar.activation(gg[:], pg[:], AF.Gelu, bias=c_zero[:])
                hv = hvp.tile([128, TT], F32, tag="hv", name="hv")
                nc.vector.scalar_tensor_tensor(
                    hv[:], p2[:, m], carry[:, m:m + 1], h2[:, m],
                    op0=OP.mult, op1=OP.add)
                y = xap.tile([128, TT + KC - 1], BF16, tag=f"xa{m}", name="y")
                nc.vector.tensor_mul(y[:, 0:TT], gg[:], hv[:])
                yt.append(y)
            ys[tt] = yt
        if step >= 1:
            tt = step - 1
            yt = ys.pop(tt)
            for q in range(TT // 128):
                pos = [ps.tile([128, 512], F32, tag="ps", name="po")
                       for _ in range(2)]
                for k in range(NE):
                    for n in range(2):
                        nc.tensor.matmul(
                            pos[n][:], yt[k][:, q * 128:(q + 1) * 128],
                            wo_sb[:, k, n * 512:(n + 1) * 512],
                            start=(k == 0), stop=(k == NE - 1))
                osb = osbp.tile([128, DIM], BF16, tag="osb", name="osb")
                for n in range(2):
                    nc.scalar.copy(osb[:, n * 512:(n + 1) * 512], pos[n][:])
                nc.scalar.dma_start(
                    out.ap()[tt * TT + q * 128:tt * TT + (q + 1) * 128, :],
                    osb[:])


_NC_CACHE = {}


def _get_nc():
    if "nc" not in _NC_CACHE:
        _NC_CACHE["nc"] = _build_kernel()
    return _NC_CACHE["nc"]


def _softplus(x):
    return np.logaddexp(0.0, x)


def kernel(x, w_in, w_conv, b_conv, w_gates, b_gates, forget_base, w_out,
           _want_trace=False):
    BF = ml_dtypes.bfloat16
    x = np.asarray(x, dtype=np.float32)
    w_in = np.asarray(w_in, dtype=np.float32)
    w_conv = np.asarray(w_conv, dtype=np.float32)
    b_conv = np.asarray(b_conv, dtype=np.float32)
    w_gates = np.asarray(w_gates, dtype=np.float32)
    b_gates = np.asarray(b_gates, dtype=np.float32)
    forget_base = np.asarray(forget_base, dtype=np.float32)
    w_out = np.asarray(w_out, dtype=np.float32)

    nc = _get_nc()

    w_in_g = np.ascontiguousarray(w_in[:E].T).astype(BF)     # [DIM, E]
    w_in_x = np.ascontiguousarray(w_in[E:].T).astype(BF)     # [DIM, E]
    w_gates_T = np.ascontiguousarray(w_gates.T).astype(BF)   # [E, 2E]
    w_out_T = np.ascontiguousarray(w_out.T).astype(BF)       # [E, DIM]
    wc_r = np.ascontiguousarray(w_conv.reshape(E, KC))
    neg_c = (-8.0 * _softplus(forget_base.astype(np.float64))).astype(
        np.float32)[:, None]

    common = {
        "w_in_g": w_in_g, "w_in_x": w_in_x, "w_gates": w_gates_T,
        "w_out": w_out_T, "wc": wc_r, "b_conv": b_conv[:, None].copy(),
        "neg_ch": 0.5 * neg_c,
        "b_fh": 0.5 * b_gates[:E, None], "b_ih": 0.5 * b_gates[E:, None],
    }
    in_maps = []
    for k in range(N_CORES):
        b, half = k // 2, k % 2
        t0 = half * T_LOC
        xT_loc = np.ascontiguousarray(x[b, t0:t0 + T_LOC, :].T).astype(BF)
        if half == 1:
            xa_halo = (x[b, t0 - (KC - 1):t0, :] @ w_in[E:].T).T
            xa_halo = np.ascontiguousarray(xa_halo).astype(BF)
        else:
            xa_halo = np.zeros((E, KC - 1), dtype=BF)
        mc = np.full((128, 1), 1.0 if half == 0 else 0.0, dtype=np.float32)
        mu = np.full((128, 1), 0.0 if half == 0 else 1.0, dtype=np.float32)
        in_maps.append({**common, "xT": xT_loc, "xa_halo": xa_halo,
                        "mask_c": mc, "mask_u": mu})

    res = run_bass_kernel_spmd(nc, in_maps, core_ids=list(range(N_CORES)),
                               trace=_want_trace)
    out_full = np.empty((B, T, DIM), dtype=np.float32)
    for k in range(N_CORES):
        b, half = k // 2, k % 2
        out_full[b, half * T_LOC:(half + 1) * T_LOC, :] = \
            res.results[k]["out"].astype(np.float32)
    if _want_trace:
        return out_full, res
    return out_full


# revision 12
# speedup vs baseline: 1.1648x; 1.0477x over previous
"""Hawk (RG-LRU) block kernel for Trainium2, SPMD over 8 NeuronCores.

Sharding: tokens. Core k handles batch b=k//2, half h=k%2 (2048 tokens).
Weights replicated, host-transposed, bf16 (full PE rate, half the HBM
traffic). Two fused passes over 4 token tiles of 512:

  pass 1: in-proj -> causal conv (DVE, bf16) -> gates matmul ->
          tanh/exp/ln activation chain (sigmoid via tanh so tanh+exp
          share one act-func table; beta via ln+exp) -> u ->
          h-scan + alpha-prefix-scan (DVE, fp32 state, bf16 out);
          h,p spill bf16 via the idle GPSIMD DMA queue.
  carry:  pairwise 4KB AllReduce moves the cross-half scan carry.
  pass 2: gate-proj + gelu, carry correction, out-proj one tile behind
          so PE never waits on the vector chain; out stored bf16.

alpha^2 runs on the (otherwise idle) GPSIMD engine. DMAs are batched
(one per tile per stream) and spread over SP/Act/Pool queues to avoid
sequencer head-of-line blocking.
"""
import sys

sys.path.insert(0, "/opt/trn_rl_repo")

import numpy as np
import ml_dtypes
from contextlib import ExitStack

import concourse.bass as bass
import concourse.tile as tile
import concourse.bacc as bacc
from concourse import mybir
from concourse.bass_utils import run_bass_kernel_spmd

F32 = mybir.dt.float32
BF16 = mybir.dt.bfloat16
AF = mybir.ActivationFunctionType
OP = mybir.AluOpType

B, T, DIM = 4, 4096, 1024
E = 1024
KC = 4
N_CORES = 8
T_LOC = T // 2
TT = 512
NTT = T_LOC // TT   # 4
NE = E // 128       # 8
NK = DIM // 128     # 8

ALPHA2_ON_POOL = True


def _build_kernel(profile_mode=False):
    nc = bacc.Bacc("TRN2", target_bir_lowering=False, debug=False,
                   num_devices=1 if profile_mode else N_CORES)

    xT = nc.dram_tensor("xT", [DIM, T_LOC], BF16, kind="ExternalInput")
    xa_halo = nc.dram_tensor("xa_halo", [E, KC - 1], BF16, kind="ExternalInput")
    w_in_g = nc.dram_tensor("w_in_g", [DIM, E], BF16, kind="ExternalInput")
    w_in_x = nc.dram_tensor("w_in_x", [DIM, E], BF16, kind="ExternalInput")
    w_gates = nc.dram_tensor("w_gates", [E, 2 * E], BF16, kind="ExternalInput")
    w_out = nc.dram_tensor("w_out", [E, DIM], BF16, kind="ExternalInput")
    wc = nc.dram_tensor("wc", [E, KC], F32, kind="ExternalInput")
    b_conv = nc.dram_tensor("b_conv", [E, 1], F32, kind="ExternalInput")
    neg_ch = nc.dram_tensor("neg_ch", [E, 1], F32, kind="ExternalInput")
    b_fh = nc.dram_tensor("b_fh", [E, 1], F32, kind="ExternalInput")
    b_ih = nc.dram_tensor("b_ih", [E, 1], F32, kind="ExternalInput")
    mask_c = nc.dram_tensor("mask_c", [128, 1], F32, kind="ExternalInput")
    mask_u = nc.dram_tensor("mask_u", [128, 1], F32, kind="ExternalInput")
    out = nc.dram_tensor("out", [T_LOC, DIM], BF16, kind="ExternalOutput")

    with tile.TileContext(nc) as tc, ExitStack() as ctx:
        _body(ctx, tc, nc, profile_mode=profile_mode,
              xT=xT, xa_halo=xa_halo, w_in_g=w_in_g, w_in_x=w_in_x,
              w_gates=w_gates, w_out=w_out, wc=wc, b_conv=b_conv,
              neg_ch=neg_ch, b_fh=b_fh, b_ih=b_ih,
              mask_c=mask_c, mask_u=mask_u, out=out)
    nc.compile()
    return nc


def _body(ctx, tc, nc, *, xT, xa_halo, w_in_g, w_in_x, w_gates, w_out, wc,
          b_conv, neg_ch, b_fh, b_ih, mask_c, mask_u, out,
          profile_mode=False):
    consts = ctx.enter_context(tc.tile_pool(name="consts", bufs=1))
    ps = ctx.enter_context(tc.tile_pool(name="ps", bufs=8, space="PSUM"))
    dram = ctx.enter_context(tc.tile_pool(name="dram", bufs=1, space="DRAM"))
    wpool = ctx.enter_context(tc.tile_pool(name="weights", bufs=1, side="right"))
    xs = ctx.enter_context(tc.tile_pool(name="xs", bufs=3))
    xap = ctx.enter_context(tc.tile_pool(name="xap", bufs=2))
    xcp = ctx.enter_context(tc.tile_pool(name="xcp", bufs=2))
    sfp = ctx.enter_context(tc.tile_pool(name="sfp", bufs=9))
    sip = ctx.enter_context(tc.tile_pool(name="sip", bufs=9))
    alp = ctx.enter_context(tc.tile_pool(name="alp", bufs=3))
    a2p = ctx.enter_context(tc.tile_pool(name="a2p", bufs=3))
    bep = ctx.enter_context(tc.tile_pool(name="bep", bufs=3))
    bsp = ctx.enter_context(tc.tile_pool(name="bsp", bufs=2))
    up = ctx.enter_context(tc.tile_pool(name="up", bufs=2))
    spl = ctx.enter_context(tc.tile_pool(name="spl", bufs=2))
    hvp = ctx.enter_context(tc.tile_pool(name="hvp", bufs=2))
    osbp = ctx.enter_context(tc.tile_pool(name="osbp", bufs=2))

    # --- constants (Act queue keeps SP free for weights/x) ---
    def chan_const(t_dram, n):
        t = consts.tile([128, NE, n], F32, tag=t_dram.name, name=t_dram.name)
        nc.scalar.dma_start(t[:], t_dram.ap().rearrange("(m p) n -> p m n", p=128))
        return t

    wc_sb = chan_const(wc, KC)
    bc_sb = chan_const(b_conv, 1)
    nch_sb = chan_const(neg_ch, 1)
    bfh_sb = chan_const(b_fh, 1)
    bih_sb = chan_const(b_ih, 1)
    mc_sb = consts.tile([128, 1], F32, tag="mc")
    nc.scalar.dma_start(mc_sb[:], mask_c.ap()[:])
    mu_sb = consts.tile([128, 1], F32, tag="mu")
    nc.scalar.dma_start(mu_sb[:], mask_u.ap()[:])
    zeros = consts.tile([128, TT], F32, tag="zeros")
    nc.vector.memset(zeros[:], 0.0)
    c_zero = consts.tile([128, 1], F32, tag="c_zero")
    nc.vector.memset(c_zero[:], 0.0)
    c_qb = consts.tile([128, 1], F32, tag="c_qb")
    nc.vector.memset(c_qb[:], 0.25000025)
    hc = consts.tile([128, NE], F32, tag="hc")
    plc = consts.tile([128, NE], F32, tag="plc")
    contrib = consts.tile([128, NE], F32, tag="contrib")
    craw = consts.tile([128, NE], F32, tag="craw")
    carry = consts.tile([128, NE], F32, tag="carry")

    h_dram = dram.tile([NTT, 128, NE, TT], BF16, tag="h_spill")
    p_dram = dram.tile([NTT, 128, NE, TT], BF16, tag="p_spill")
    cc_in = dram.tile([E], F32, tag="cc_in")
    cc_out = dram.tile([E], F32, tag="cc_out")

    # --- weights (persistent bf16; w_out shares the w_in_x slot) ---
    wx_sb = wpool.tile([128, NK, E], BF16, tag="wxo", name="wx_sb")
    wg_sb = wpool.tile([128, NK, 2 * E], BF16, tag="wg", name="wg_sb")
    wgi_sb = wpool.tile([128, NK, E], BF16, tag="wgi", name="wgi_sb")
    wx_src = w_in_x.ap().rearrange("(k p) e -> p k e", p=128)
    wg_src = w_gates.ap().rearrange("(k p) f -> p k f", p=128)
    wgi_src = w_in_g.ap().rearrange("(k p) e -> p k e", p=128)
    wo_src = w_out.ap().rearrange("(k p) c -> p k c", p=128)
    xT_r = xT.ap().rearrange("(k p) t -> p k t", p=128)
    halo_r = xa_halo.ap().rearrange("(m p) n -> p m n", p=128)

    def load_x_tile(tt):
        t = xs.tile([128, NK, TT], BF16, tag="xstream", name="xt")
        nc.sync.dma_start(t[:], xT_r[:, :, tt * TT:(tt + 1) * TT])
        return t

    # ================= pass 1: xa proj + conv + gates ==================
    # The u/scan stage for tile tt runs one tile deferred (during tt+1)
    # so the next tile's conv is first in the DVE queue and PE never
    # waits on the scan tail.
    def deferred_act(st):
        tt, sfs, sis, xcs = st
        als, bes = {}, {}
        for m in range(NE):  # alpha = exp(-c/2*vf - c/2)  [same table as tanh]
            al = alp.tile([128, TT], F32, tag="al", name="al")
            nc.scalar.activation(al[:], sfs[m][:], AF.Exp,
                                 scale=nch_sb[:, m, 0:1],
                                 bias=nch_sb[:, m, 0:1])
            als[m] = al
            a2 = a2p.tile([128, TT], F32, tag="a2", name="a2")
            nc.gpsimd.tensor_mul(a2[:], al[:], al[:])
            bes[m] = a2
        for m in range(NE):  # beta/2 = sqrt(0.25000025 - 0.25*alpha^2)
            lnb = bep.tile([128, TT], F32, tag="be", name="lnb")
            nc.scalar.activation(lnb[:], bes[m][:], AF.Ln,
                                 scale=-0.25, bias=c_qb[:])
            nc.scalar.activation(lnb[:], lnb[:], AF.Exp, scale=0.5,
                                 bias=c_zero[:])
            bes[m] = lnb
        return als, bes

    def deferred_dve(st, als, bes):
        tt, sfs, sis, xcs = st
        h_all = spl.tile([128, NE, TT], BF16, tag="hall", name="h_all")
        p_all = spl.tile([128, NE, TT], BF16, tag="pall", name="p_all")
        for m in range(NE):
            bs = bsp.tile([128, TT], F32, tag="bs", name="bs")
            nc.vector.scalar_tensor_tensor(bs[:], sis[m][:], 1.0, bes[m][:],
                                           op0=OP.add, op1=OP.mult)
            u = up.tile([128, TT], F32, tag="u", name="u")
            nc.vector.tensor_mul(u[:], bs[:], xcs[m][:])
            nc.vector.tensor_tensor_scan(
                h_all[:, m], als[m][:], u[:],
                0.0 if tt == 0 else hc[:, m:m + 1],
                op0=OP.mult, op1=OP.add)
            nc.vector.tensor_copy(hc[:, m:m + 1], h_all[:, m, TT - 1:TT])
            nc.vector.tensor_tensor_scan(
                p_all[:, m], als[m][:], zeros[:],
                1.0 if tt == 0 else plc[:, m:m + 1],
                op0=OP.mult, op1=OP.add)
            nc.vector.tensor_copy(plc[:, m:m + 1], p_all[:, m, TT - 1:TT])
        nc.gpsimd.dma_start(h_dram[tt], h_all[:])
        nc.gpsimd.dma_start(p_dram[tt], p_all[:])

    prev_xa = None
    pending = None
    xt_p2 = None
    for tt in range(NTT):
        if pending is not None:
            d_als, d_bes = deferred_act(pending)
        if tt == 0:
            xt = xs.tile([128, NK, TT], BF16, tag="xstream", name="xt")
            for k in range(NK):
                nc.sync.dma_start(wx_sb[:, k], wx_src[:, k])
                nc.sync.dma_start(xt[:, k], xT_r[:, k, 0:TT])
            for k in range(NK):
                nc.sync.dma_start(wg_sb[:, k], wg_src[:, k])
        else:
            xt = load_x_tile(tt)

        xas, xcs = [], []
        for m in range(NE):
            pa = ps.tile([128, TT], F32, tag="ps", name="pa")
            for k in range(NK):
                nc.tensor.matmul(pa[:], wx_sb[:, k, m * 128:(m + 1) * 128],
                                 xt[:, k], start=(k == 0), stop=(k == NK - 1))
            xa = xap.tile([128, TT + KC - 1], BF16, tag=f"xa{m}", name="xa")
            nc.scalar.copy(xa[:, KC - 1:TT + KC - 1], pa[:])
            if tt == 0:
                nc.scalar.dma_start(xa[:, 0:KC - 1], halo_r[:, m])
            else:
                nc.vector.tensor_copy(xa[:, 0:KC - 1],
                                      prev_xa[m][:, TT:TT + KC - 1])
            xc = xcp.tile([128, TT], BF16, tag=f"xc{m}", name="xc")
            nc.vector.tensor_scalar(
                xc[:], xa[:, 0:TT], wc_sb[:, m, 0:1], bc_sb[:, m, 0:1],
                op0=OP.mult, op1=OP.add)
            for j in range(1, KC):
                nc.vector.scalar_tensor_tensor(
                    xc[:], xa[:, j:j + TT], wc_sb[:, m, j:j + 1],
                    xc[:], op0=OP.mult, op1=OP.add)
            xas.append(xa)
            xcs.append(xc)
        if pending is not None:
            deferred_dve(pending, d_als, d_bes)

        sfs, sis = {}, {}
        for g in range(2):
            ms = range(g * 4, g * 4 + 4)
            pfs, pis = {}, {}
            for m in ms:
                pf = ps.tile([128, TT], F32, tag="ps", name="pf")
                for k in range(NK):
                    nc.tensor.matmul(pf[:], wg_sb[:, k, m * 128:(m + 1) * 128],
                                     xcs[k][:], start=(k == 0), stop=(k == NK - 1))
                pfs[m] = pf
                pi = ps.tile([128, TT], F32, tag="ps", name="pi")
                for k in range(NK):
                    nc.tensor.matmul(pi[:], wg_sb[:, k, E + m * 128:E + (m + 1) * 128],
                                     xcs[k][:], start=(k == 0), stop=(k == NK - 1))
                pis[m] = pi
            for m in ms:  # sigmoid(x) = 0.5*tanh(x/2)+0.5, folded downstream
                sf = sfp.tile([128, TT], BF16, tag="sf", name="sf")
                nc.scalar.activation(sf[:], pfs[m][:], AF.Tanh,
                                     scale=0.5, bias=bfh_sb[:, m, 0:1])
                sfs[m] = sf
                si = sip.tile([128, TT], BF16, tag="si", name="si")
                nc.scalar.activation(si[:], pis[m][:], AF.Tanh,
                                     scale=0.5, bias=bih_sb[:, m, 0:1])
                sis[m] = si

        for k in (2 * tt, 2 * tt + 1):
            nc.sync.dma_start(wgi_sb[:, k], wgi_src[:, k])
        prev_xa = xas
        pending = (tt, sfs, sis, xcs)
        if tt == NTT - 1:
            xt_p2 = load_x_tile(0)

    d_als, d_bes = deferred_act(pending)
    deferred_dve(pending, d_als, d_bes)
    pending = None

    # w_out loads into the (now dead) w_in_x slot
    wo_sb = wpool.tile([128, NK, DIM], BF16, tag="wxo", name="wo_sb")
    for k in range(NK):
        nc.sync.dma_start(wo_sb[:, k], wo_src[:, k])

    # ====== pass 2 prefill: first two gate projections (carry-free) ====
    def gate_phase(tt, xt):
        h2 = spl.tile([128, NE, TT], BF16, tag="hall", name="h2")
        nc.sync.dma_start(h2[:], h_dram[tt])
        p2 = spl.tile([128, NE, TT], BF16, tag="pall", name="p2")
        nc.sync.dma_start(p2[:], p_dram[tt])
        ggs = []
        for m in range(NE):
            pg = ps.tile([128, TT], F32, tag="ps", name="pg")
            for k in range(NK):
                nc.tensor.matmul(pg[:], wgi_sb[:, k, m * 128:(m + 1) * 128],
                                 xt[:, k], start=(k == 0), stop=(k == NK - 1))
            gg = xcp.tile([128, TT], BF16, tag=f"xc{m}", name="gg")
            nc.scalar.activation(gg[:], pg[:], AF.Gelu, bias=c_zero[:])
            ggs.append(gg)
        return h2, p2, ggs

    def y_phase(tt, st):
        h2, p2, ggs = st
        yt = []
        for m in range(NE):
            hv = hvp.tile([128, TT], F32, tag="hv", name="hv")
            nc.vector.scalar_tensor_tensor(
                hv[:], p2[:, m], carry[:, m:m + 1], h2[:, m],
                op0=OP.mult, op1=OP.add)
            y = xap.tile([128, TT + KC - 1], BF16, tag=f"xa{m}", name="y")
            nc.vector.tensor_mul(y[:, 0:TT], ggs[m][:], hv[:])
            yt.append(y)
        return yt

    def out_phase(tt, yt):
        for q in range(TT // 128):
            pos = [ps.tile([128, 512], F32, tag="ps", name="po")
                   for _ in range(2)]
            for k in range(NE):
                for n in range(2):
                    nc.tensor.matmul(
                        pos[n][:], yt[k][:, q * 128:(q + 1) * 128],
                        wo_sb[:, k, n * 512:(n + 1) * 512],
                        start=(k == 0), stop=(k == NE - 1))
            osb = osbp.tile([128, DIM], BF16, tag="osb", name="osb")
            for n in range(2):
                nc.scalar.copy(osb[:, n * 512:(n + 1) * 512], pos[n][:])
            nc.scalar.dma_start(
                out.ap()[tt * TT + q * 128:tt * TT + (q + 1) * 128, :],
                osb[:])

    gstates = {}
    gstates[0] = gate_phase(0, xt_p2)
    gstates[1] = gate_phase(1, load_x_tile(1))

    # ================= carry exchange (pairwise AllReduce, 4KB) ========
    nc.vector.tensor_scalar(contrib[:], hc[:], mc_sb[:, 0:1], None,
                            op0=OP.mult)
    nc.sync.dma_start(cc_in[:].rearrange("(j p) -> p j", p=128), contrib[:])
    if profile_mode:
        nc.sync.dma_start(cc_out[:], cc_in[:])
    else:
        nc.gpsimd.collective_compute(
            "AllReduce", OP.add,
            replica_groups=[[0, 1], [2, 3], [4, 5], [6, 7]],
            ins=[cc_in[:].opt()], outs=[cc_out[:].opt()])
    nc.sync.dma_start(craw[:], cc_out[:].rearrange("(j p) -> p j", p=128))
    nc.vector.tensor_scalar(carry[:], craw[:], mu_sb[:, 0:1], None,
                            op0=OP.mult)

    # ================= pass 2: correction + out proj ===================
    for tt in range(NTT):
        yt = y_phase(tt, gstates.pop(tt))
        if tt + 2 < NTT:
            gstates[tt + 2] = gate_phase(tt + 2, load_x_tile(tt + 2))
        out_phase(tt, yt)


_NC_CACHE = {}


def _get_nc():
    if "nc" not in _NC_CACHE:
        _NC_CACHE["nc"] = _build_kernel()
    return _NC_CACHE["nc"]


def _softplus(x):
    return np.logaddexp(0.0, x)


def kernel(x, w_in, w_conv, b_conv, w_gates, b_gates, forget_base, w_out,
           _want_trace=False):
    BF = ml_dtypes.bfloat16
    x = np.asarray(x, dtype=np.float32)
    w_in = np.asarray(w_in, dtype=np.float32)
    w_conv = np.asarray(w_conv, dtype=np.float32)
    b_conv = np.asarray(b_conv, dtype=np.float32)
    w_gates = np.asarray(w_gates, dtype=np.float32)
    b_gates = np.asarray(b_gates, dtype=np.float32)
    forget_base = np.asarray(forget_base, dtype=np.float32)
    w_out = np.asarray(w_out, dtype=np.float32)

    nc = _get_nc()

    w_in_g = np.ascontiguousarray(w_in[:E].T).astype(BF)     # [DIM, E]
    w_in_x = np.ascontiguousarray(w_in[E:].T).astype(BF)     # [DIM, E]
    w_gates_T = np.ascontiguousarray(w_gates.T).astype(BF)   # [E, 2E]
    w_out_T = np.ascontiguousarray(w_out.T).astype(BF)       # [E, DIM]
    wc_r = np.ascontiguousarray(w_conv.reshape(E, KC))
    neg_c = (-8.0 * _softplus(forget_base.astype(np.float64))).astype(
        np.float32)[:, None]

    common = {
        "w_in_g": w_in_g, "w_in_x": w_in_x, "w_gates": w_gates_T,
        "w_out": w_out_T, "wc": wc_r, "b_conv": b_conv[:, None].copy(),
        "neg_ch": 0.5 * neg_c,
        "b_fh": 0.5 * b_gates[:E, None], "b_ih": 0.5 * b_gates[E:, None],
    }
    in_maps = []
    for k in range(N_CORES):
        b, half = k // 2, k % 2
        t0 = half * T_LOC
        xT_loc = np.ascontiguousarray(x[b, t0:t0 + T_LOC, :].T).astype(BF)
        if half == 1:
            xa_halo = (x[b, t0 - (KC - 1):t0, :] @ w_in[E:].T).T
            xa_halo = np.ascontiguousarray(xa_halo).astype(BF)
        else:
            xa_halo = np.zeros((E, KC - 1), dtype=BF)
        mc = np.full((128, 1), 1.0 if half == 0 else 0.0, dtype=np.float32)
        mu = np.full((128, 1), 0.0 if half == 0 else 1.0, dtype=np.float32)
        in_maps.append({**common, "xT": xT_loc, "xa_halo": xa_halo,
                        "mask_c": mc, "mask_u": mu})

    res = run_bass_kernel_spmd(nc, in_maps, core_ids=list(range(N_CORES)),
                               trace=_want_trace)
    out_full = np.empty((B, T, DIM), dtype=np.float32)
    for k in range(N_CORES):
        b, half = k // 2, k % 2
        out_full[b, half * T_LOC:(half + 1) * T_LOC, :] = \
            res.results[k]["out"].astype(np.float32)
    if _want_trace:
        return out_full, res
    return out_full


# revision 13
# speedup vs baseline: 1.1708x; 1.0051x over previous
"""Hawk (RG-LRU) block kernel for Trainium2, SPMD over 8 NeuronCores.

Sharding: tokens. Core k handles batch b=k//2, half h=k%2 (2048 tokens).
Weights replicated, host-transposed, bf16 (full PE rate, half the HBM
traffic). Two fused passes over 4 token tiles of 512:

  pass 1: in-proj -> causal conv (DVE, bf16) -> gates matmul ->
          tanh/exp/ln activation chain (sigmoid via tanh so tanh+exp
          share one act-func table; beta via ln+exp) -> u ->
          h-scan + alpha-prefix-scan (DVE, fp32 state, bf16 out);
          h,p spill bf16 via the idle GPSIMD DMA queue.
  carry:  pairwise 4KB AllReduce moves the cross-half scan carry.
  pass 2: gate-proj + gelu, carry correction, out-proj one tile behind
          so PE never waits on the vector chain; out stored bf16.

alpha^2 runs on the (otherwise idle) GPSIMD engine. DMAs are batched
(one per tile per stream) and spread over SP/Act/Pool queues to avoid
sequencer head-of-line blocking.
"""
import sys

sys.path.insert(0, "/opt/trn_rl_repo")

import numpy as np
import ml_dtypes
from contextlib import ExitStack

import concourse.bass as bass
import concourse.tile as tile
import concourse.bacc as bacc
from concourse import mybir
from concourse.bass_utils import run_bass_kernel_spmd

F32 = mybir.dt.float32
BF16 = mybir.dt.bfloat16
AF = mybir.ActivationFunctionType
OP = mybir.AluOpType

B, T, DIM = 4, 4096, 1024
E = 1024
KC = 4
N_CORES = 8
T_LOC = T // 2
TT = 512
NTT = T_LOC // TT   # 4
NE = E // 128       # 8
NK = DIM // 128     # 8

ALPHA2_ON_POOL = True


def _build_kernel(profile_mode=False):
    nc = bacc.Bacc("TRN2", target_bir_lowering=False, debug=False,
                   num_devices=1 if profile_mode else N_CORES)

    xT = nc.dram_tensor("xT", [DIM, T_LOC], BF16, kind="ExternalInput")
    xa_halo = nc.dram_tensor("xa_halo", [E, KC - 1], BF16, kind="ExternalInput")
    w_in_g = nc.dram_tensor("w_in_g", [DIM, E], BF16, kind="ExternalInput")
    w_in_x = nc.dram_tensor("w_in_x", [DIM, E], BF16, kind="ExternalInput")
    w_gates = nc.dram_tensor("w_gates", [E, 2 * E], BF16, kind="ExternalInput")
    w_out = nc.dram_tensor("w_out", [E, DIM], BF16, kind="ExternalInput")
    wc = nc.dram_tensor("wc", [E, KC], F32, kind="ExternalInput")
    b_conv = nc.dram_tensor("b_conv", [E, 1], F32, kind="ExternalInput")
    neg_ch = nc.dram_tensor("neg_ch", [E, 1], F32, kind="ExternalInput")
    b_fh = nc.dram_tensor("b_fh", [E, 1], F32, kind="ExternalInput")
    b_ih = nc.dram_tensor("b_ih", [E, 1], F32, kind="ExternalInput")
    mask_c = nc.dram_tensor("mask_c", [128, 1], F32, kind="ExternalInput")
    mask_u = nc.dram_tensor("mask_u", [128, 1], F32, kind="ExternalInput")
    out = nc.dram_tensor("out", [T_LOC, DIM], BF16, kind="ExternalOutput")

    with tile.TileContext(nc) as tc, ExitStack() as ctx:
        _body(ctx, tc, nc, profile_mode=profile_mode,
              xT=xT, xa_halo=xa_halo, w_in_g=w_in_g, w_in_x=w_in_x,
              w_gates=w_gates, w_out=w_out, wc=wc, b_conv=b_conv,
              neg_ch=neg_ch, b_fh=b_fh, b_ih=b_ih,
              mask_c=mask_c, mask_u=mask_u, out=out)
    nc.compile()
    return nc


def _body(ctx, tc, nc, *, xT, xa_halo, w_in_g, w_in_x, w_gates, w_out, wc,
          b_conv, neg_ch, b_fh, b_ih, mask_c, mask_u, out,
          profile_mode=False):
    consts = ctx.enter_context(tc.tile_pool(name="consts", bufs=1))
    ps = ctx.enter_context(tc.tile_pool(name="ps", bufs=8, space="PSUM"))
    dram = ctx.enter_context(tc.tile_pool(name="dram", bufs=1, space="DRAM"))
    wpool = ctx.enter_context(tc.tile_pool(name="weights", bufs=1, side="right"))
    xs = ctx.enter_context(tc.tile_pool(name="xs", bufs=3))
    xap = ctx.enter_context(tc.tile_pool(name="xap", bufs=2))
    xcp = ctx.enter_context(tc.tile_pool(name="xcp", bufs=2))
    sfp = ctx.enter_context(tc.tile_pool(name="sfp", bufs=9))
    sip = ctx.enter_context(tc.tile_pool(name="sip", bufs=9))
    alp = ctx.enter_context(tc.tile_pool(name="alp", bufs=3))
    a2p = ctx.enter_context(tc.tile_pool(name="a2p", bufs=3))
    bep = ctx.enter_context(tc.tile_pool(name="bep", bufs=3))
    bsp = ctx.enter_context(tc.tile_pool(name="bsp", bufs=2))
    up = ctx.enter_context(tc.tile_pool(name="up", bufs=2))
    spl = ctx.enter_context(tc.tile_pool(name="spl", bufs=2))
    hvp = ctx.enter_context(tc.tile_pool(name="hvp", bufs=2))
    osbp = ctx.enter_context(tc.tile_pool(name="osbp", bufs=2))

    # --- constants (Act queue keeps SP free for weights/x) ---
    def chan_const(t_dram, n):
        t = consts.tile([128, NE, n], F32, tag=t_dram.name, name=t_dram.name)
        nc.scalar.dma_start(t[:], t_dram.ap().rearrange("(m p) n -> p m n", p=128))
        return t

    wc_sb = chan_const(wc, KC)
    bc_sb = chan_const(b_conv, 1)
    nch_sb = chan_const(neg_ch, 1)
    bfh_sb = chan_const(b_fh, 1)
    bih_sb = chan_const(b_ih, 1)
    mc_sb = consts.tile([128, 1], F32, tag="mc")
    nc.scalar.dma_start(mc_sb[:], mask_c.ap()[:])
    mu_sb = consts.tile([128, 1], F32, tag="mu")
    nc.scalar.dma_start(mu_sb[:], mask_u.ap()[:])
    zeros = consts.tile([128, TT], F32, tag="zeros")
    nc.vector.memset(zeros[:], 0.0)
    c_zero = consts.tile([128, 1], F32, tag="c_zero")
    nc.vector.memset(c_zero[:], 0.0)
    c_qb = consts.tile([128, 1], F32, tag="c_qb")
    nc.vector.memset(c_qb[:], 0.25000025)
    hc = consts.tile([128, NE], F32, tag="hc")
    plc = consts.tile([128, NE], F32, tag="plc")
    contrib = consts.tile([128, NE], F32, tag="contrib")
    craw = consts.tile([128, NE], F32, tag="craw")
    carry = consts.tile([128, NE], F32, tag="carry")

    h_dram = dram.tile([NTT, 128, NE, TT], BF16, tag="h_spill")
    p_dram = dram.tile([NTT, 128, NE, TT], BF16, tag="p_spill")
    cc_in = dram.tile([E], F32, tag="cc_in")
    cc_out = dram.tile([E], F32, tag="cc_out")

    # --- weights (persistent bf16; w_out shares the w_in_x slot) ---
    wx_sb = wpool.tile([128, NK, E], BF16, tag="wxo", name="wx_sb")
    wg_sb = wpool.tile([128, NK, 2 * E], BF16, tag="wg", name="wg_sb")
    wgi_sb = wpool.tile([128, NK, E], BF16, tag="wgi", name="wgi_sb")
    wx_src = w_in_x.ap().rearrange("(k p) e -> p k e", p=128)
    wg_src = w_gates.ap().rearrange("(k p) f -> p k f", p=128)
    wgi_src = w_in_g.ap().rearrange("(k p) e -> p k e", p=128)
    wo_src = w_out.ap().rearrange("(k p) c -> p k c", p=128)
    xT_r = xT.ap().rearrange("(k p) t -> p k t", p=128)
    halo_r = xa_halo.ap().rearrange("(m p) n -> p m n", p=128)

    def load_x_tile(tt):
        t = xs.tile([128, NK, TT], BF16, tag="xstream", name="xt")
        nc.sync.dma_start(t[:], xT_r[:, :, tt * TT:(tt + 1) * TT])
        return t

    # ================= pass 1: xa proj + conv + gates ==================
    # The u/scan stage for tile tt runs one tile deferred (during tt+1)
    # so the next tile's conv is first in the DVE queue and PE never
    # waits on the scan tail.
    def deferred_act(st):
        tt, sfs, sis, xcs = st
        als, bes = {}, {}
        for m in range(NE):  # alpha = exp(-c/2*vf - c/2)  [same table as tanh]
            al = alp.tile([128, TT], F32, tag="al", name="al")
            nc.scalar.activation(al[:], sfs[m][:], AF.Exp,
                                 scale=nch_sb[:, m, 0:1],
                                 bias=nch_sb[:, m, 0:1])
            als[m] = al
            a2 = a2p.tile([128, TT], F32, tag="a2", name="a2")
            nc.gpsimd.tensor_mul(a2[:], al[:], al[:])
            bes[m] = a2
        for m in range(NE):  # beta/2 = sqrt(0.25000025 - 0.25*alpha^2)
            be = bep.tile([128, TT], F32, tag="be", name="be")
            nc.scalar.activation(be[:], bes[m][:], AF.Sqrt,
                                 scale=-0.25, bias=c_qb[:])
            bes[m] = be
        return als, bes

    def deferred_dve(st, als, bes):
        tt, sfs, sis, xcs = st
        h_all = spl.tile([128, NE, TT], BF16, tag="hall", name="h_all")
        p_all = spl.tile([128, NE, TT], BF16, tag="pall", name="p_all")
        for m in range(NE):
            bs = bsp.tile([128, TT], F32, tag="bs", name="bs")
            nc.vector.scalar_tensor_tensor(bs[:], sis[m][:], 1.0, bes[m][:],
                                           op0=OP.add, op1=OP.mult)
            u = up.tile([128, TT], F32, tag="u", name="u")
            nc.vector.tensor_mul(u[:], bs[:], xcs[m][:])
            nc.vector.tensor_tensor_scan(
                h_all[:, m], als[m][:], u[:],
                0.0 if tt == 0 else hc[:, m:m + 1],
                op0=OP.mult, op1=OP.add)
            nc.vector.tensor_copy(hc[:, m:m + 1], h_all[:, m, TT - 1:TT])
            nc.vector.tensor_tensor_scan(
                p_all[:, m], als[m][:], zeros[:],
                1.0 if tt == 0 else plc[:, m:m + 1],
                op0=OP.mult, op1=OP.add)
            nc.vector.tensor_copy(plc[:, m:m + 1], p_all[:, m, TT - 1:TT])
        nc.gpsimd.dma_start(h_dram[tt], h_all[:])
        nc.gpsimd.dma_start(p_dram[tt], p_all[:])

    prev_xa = None
    pending = None
    xt_p2 = None
    for tt in range(NTT):
        if pending is not None:
            d_als, d_bes = deferred_act(pending)
        if tt == 0:
            xt = xs.tile([128, NK, TT], BF16, tag="xstream", name="xt")
            for k in range(NK):
                nc.sync.dma_start(wx_sb[:, k], wx_src[:, k])
                nc.sync.dma_start(xt[:, k], xT_r[:, k, 0:TT])
            for k in range(NK):
                nc.sync.dma_start(wg_sb[:, k], wg_src[:, k])
        else:
            xt = load_x_tile(tt)

        xas, xcs = [], []
        for m in range(NE):
            pa = ps.tile([128, TT], F32, tag="ps", name="pa")
            for k in range(NK):
                nc.tensor.matmul(pa[:], wx_sb[:, k, m * 128:(m + 1) * 128],
                                 xt[:, k], start=(k == 0), stop=(k == NK - 1))
            xa = xap.tile([128, TT + KC - 1], BF16, tag=f"xa{m}", name="xa")
            nc.scalar.copy(xa[:, KC - 1:TT + KC - 1], pa[:])
            if tt == 0:
                nc.scalar.dma_start(xa[:, 0:KC - 1], halo_r[:, m])
            else:
                nc.vector.tensor_copy(xa[:, 0:KC - 1],
                                      prev_xa[m][:, TT:TT + KC - 1])
            xc = xcp.tile([128, TT], BF16, tag=f"xc{m}", name="xc")
            nc.vector.tensor_scalar(
                xc[:], xa[:, 0:TT], wc_sb[:, m, 0:1], bc_sb[:, m, 0:1],
                op0=OP.mult, op1=OP.add)
            for j in range(1, KC):
                nc.vector.scalar_tensor_tensor(
                    xc[:], xa[:, j:j + TT], wc_sb[:, m, j:j + 1],
                    xc[:], op0=OP.mult, op1=OP.add)
            xas.append(xa)
            xcs.append(xc)
        if pending is not None:
            deferred_dve(pending, d_als, d_bes)

        sfs, sis = {}, {}
        for g in range(2):
            ms = range(g * 4, g * 4 + 4)
            pfs, pis = {}, {}
            for m in ms:
                pf = ps.tile([128, TT], F32, tag="ps", name="pf")
                for k in range(NK):
                    nc.tensor.matmul(pf[:], wg_sb[:, k, m * 128:(m + 1) * 128],
                                     xcs[k][:], start=(k == 0), stop=(k == NK - 1))
                pfs[m] = pf
                pi = ps.tile([128, TT], F32, tag="ps", name="pi")
                for k in range(NK):
                    nc.tensor.matmul(pi[:], wg_sb[:, k, E + m * 128:E + (m + 1) * 128],
                                     xcs[k][:], start=(k == 0), stop=(k == NK - 1))
                pis[m] = pi
            for m in ms:  # sigmoid(x) = 0.5*tanh(x/2)+0.5, folded downstream
                sf = sfp.tile([128, TT], BF16, tag="sf", name="sf")
                nc.scalar.activation(sf[:], pfs[m][:], AF.Tanh,
                                     scale=0.5, bias=bfh_sb[:, m, 0:1])
                sfs[m] = sf
                si = sip.tile([128, TT], BF16, tag="si", name="si")
                nc.scalar.activation(si[:], pis[m][:], AF.Tanh,
                                     scale=0.5, bias=bih_sb[:, m, 0:1])
                sis[m] = si

        for k in (2 * tt, 2 * tt + 1):
            nc.sync.dma_start(wgi_sb[:, k], wgi_src[:, k])
        prev_xa = xas
        pending = (tt, sfs, sis, xcs)
        if tt == NTT - 1:
            xt_p2 = load_x_tile(0)

    d_als, d_bes = deferred_act(pending)
    deferred_dve(pending, d_als, d_bes)
    pending = None

    # w_out loads into the (now dead) w_in_x slot
    wo_sb = wpool.tile([128, NK, DIM], BF16, tag="wxo", name="wo_sb")
    for k in range(NK):
        nc.sync.dma_start(wo_sb[:, k], wo_src[:, k])

    # ====== pass 2 prefill: first two gate projections (carry-free) ====
    def gate_phase(tt, xt):
        h2 = spl.tile([128, NE, TT], BF16, tag="hall", name="h2")
        nc.sync.dma_start(h2[:], h_dram[tt])
        p2 = spl.tile([128, NE, TT], BF16, tag="pall", name="p2")
        nc.sync.dma_start(p2[:], p_dram[tt])
        ggs = []
        for m in range(NE):
            pg = ps.tile([128, TT], F32, tag="ps", name="pg")
            for k in range(NK):
                nc.tensor.matmul(pg[:], wgi_sb[:, k, m * 128:(m + 1) * 128],
                                 xt[:, k], start=(k == 0), stop=(k == NK - 1))
            gg = xcp.tile([128, TT], BF16, tag=f"xc{m}", name="gg")
            nc.scalar.activation(gg[:], pg[:], AF.Gelu, bias=c_zero[:])
            ggs.append(gg)
        return h2, p2, ggs

    def y_phase(tt, st):
        h2, p2, ggs = st
        yt = []
        for m in range(NE):
            hv = hvp.tile([128, TT], F32, tag="hv", name="hv")
            nc.vector.scalar_tensor_tensor(
                hv[:], p2[:, m], carry[:, m:m + 1], h2[:, m],
                op0=OP.mult, op1=OP.add)
            y = xap.tile([128, TT + KC - 1], BF16, tag=f"xa{m}", name="y")
            nc.vector.tensor_mul(y[:, 0:TT], ggs[m][:], hv[:])
            yt.append(y)
        return yt

    def out_phase(tt, yt):
        for q in range(TT // 128):
            pos = [ps.tile([128, 512], F32, tag="ps", name="po")
                   for _ in range(2)]
            for k in range(NE):
                for n in range(2):
                    nc.tensor.matmul(
                        pos[n][:], yt[k][:, q * 128:(q + 1) * 128],
                        wo_sb[:, k, n * 512:(n + 1) * 512],
                        start=(k == 0), stop=(k == NE - 1))
            osb = osbp.tile([128, DIM], BF16, tag="osb", name="osb")
            for n in range(2):
                nc.scalar.copy(osb[:, n * 512:(n + 1) * 512], pos[n][:])
            nc.scalar.dma_start(
                out.ap()[tt * TT + q * 128:tt * TT + (q + 1) * 128, :],
                osb[:])

    gstates = {}
    gstates[0] = gate_phase(0, xt_p2)
    gstates[1] = gate_phase(1, load_x_tile(1))

    # ================= carry exchange (pairwise AllReduce, 4KB) ========
    nc.vector.tensor_scalar(contrib[:], hc[:], mc_sb[:, 0:1], None,
                            op0=OP.mult)
    nc.sync.dma_start(cc_in[:].rearrange("(j p) -> p j", p=128), contrib[:])
    if profile_mode:
        nc.sync.dma_start(cc_out[:], cc_in[:])
    else:
        nc.gpsimd.collective_compute(
            "AllReduce", OP.add,
            replica_groups=[[0, 1], [2, 3], [4, 5], [6, 7]],
            ins=[cc_in[:].opt()], outs=[cc_out[:].opt()])
    nc.sync.dma_start(craw[:], cc_out[:].rearrange("(j p) -> p j", p=128))
    nc.vector.tensor_scalar(carry[:], craw[:], mu_sb[:, 0:1], None,
                            op0=OP.mult)

    # ================= pass 2: correction + out proj ===================
    for tt in range(NTT):
        yt = y_phase(tt, gstates.pop(tt))
        if tt + 2 < NTT:
            gstates[tt + 2] = gate_phase(tt + 2, load_x_tile(tt + 2))
        out_phase(tt, yt)


_NC_CACHE = {}


def _get_nc():
    if "nc" not in _NC_CACHE:
        _NC_CACHE["nc"] = _build_kernel()
    return _NC_CACHE["nc"]


def _softplus(x):
    return np.logaddexp(0.0, x)


def kernel(x, w_in, w_conv, b_conv, w_gates, b_gates, forget_base, w_out,
           _want_trace=False):
    BF = ml_dtypes.bfloat16
    x = np.asarray(x, dtype=np.float32)
    w_in = np.asarray(w_in, dtype=np.float32)
    w_conv = np.asarray(w_conv, dtype=np.float32)
    b_conv = np.asarray(b_conv, dtype=np.float32)
    w_gates = np.asarray(w_gates, dtype=np.float32)
    b_gates = np.asarray(b_gates, dtype=np.float32)
    forget_base = np.asarray(forget_base, dtype=np.float32)
    w_out = np.asarray(w_out, dtype=np.float32)

    nc = _get_nc()

    w_in_g = np.ascontiguousarray(w_in[:E].T).astype(BF)     # [DIM, E]
    w_in_x = np.ascontiguousarray(w_in[E:].T).astype(BF)     # [DIM, E]
    w_gates_T = np.ascontiguousarray(w_gates.T).astype(BF)   # [E, 2E]
    w_out_T = np.ascontiguousarray(w_out.T).astype(BF)       # [E, DIM]
    wc_r = np.ascontiguousarray(w_conv.reshape(E, KC))
    neg_c = (-8.0 * _softplus(forget_base.astype(np.float64))).astype(
        np.float32)[:, None]

    common = {
        "w_in_g": w_in_g, "w_in_x": w_in_x, "w_gates": w_gates_T,
        "w_out": w_out_T, "wc": wc_r, "b_conv": b_conv[:, None].copy(),
        "neg_ch": 0.5 * neg_c,
        "b_fh": 0.5 * b_gates[:E, None], "b_ih": 0.5 * b_gates[E:, None],
    }
    in_maps = []
    for k in range(N_CORES):
        b, half = k // 2, k % 2
        t0 = half * T_LOC
        xT_loc = np.ascontiguousarray(x[b, t0:t0 + T_LOC, :].T).astype(BF)
        if half == 1:
            xa_halo = (x[b, t0 - (KC - 1):t0, :] @ w_in[E:].T).T
            xa_halo = np.ascontiguousarray(xa_halo).astype(BF)
        else:
            xa_halo = np.zeros((E, KC - 1), dtype=BF)
        mc = np.full((128, 1), 1.0 if half == 0 else 0.0, dtype=np.float32)
        mu = np.full((128, 1), 0.0 if half == 0 else 1.0, dtype=np.float32)
        in_maps.append({**common, "xT": xT_loc, "xa_halo": xa_halo,
                        "mask_c": mc, "mask_u": mu})

    res = run_bass_kernel_spmd(nc, in_maps, core_ids=list(range(N_CORES)),
                               trace=_want_trace)
    out_full = np.empty((B, T, DIM), dtype=np.float32)
    for k in range(N_CORES):
        b, half = k // 2, k % 2
        out_full[b, half * T_LOC:(half + 1) * T_LOC, :] = \
            res.results[k]["out"].astype(np.float32)
    if _want_trace:
        return out_full, res
    return out_full


# revision 14
# speedup vs baseline: 1.3210x; 1.1283x over previous
"""Hawk (RG-LRU) block kernel for Trainium2, SPMD over 8 NeuronCores.

Sharding: tokens. Core k handles batch b=k//2, half h=k%2 (2048 tokens).
Weights replicated, host-transposed, bf16 (full PE rate, half the HBM
traffic). Two fused passes over 4 token tiles of 512:

  pass 1: in-proj -> causal conv (DVE, bf16) -> gates matmul ->
          tanh/exp/ln activation chain (sigmoid via tanh so tanh+exp
          share one act-func table; beta via ln+exp) -> u ->
          h-scan + alpha-prefix-scan (DVE, fp32 state, bf16 out);
          h,p spill bf16 via the idle GPSIMD DMA queue.
  carry:  pairwise 4KB AllReduce moves the cross-half scan carry.
  pass 2: gate-proj + gelu, carry correction, out-proj one tile behind
          so PE never waits on the vector chain; out stored bf16.

alpha^2 runs on the (otherwise idle) GPSIMD engine. DMAs are batched
(one per tile per stream) and spread over SP/Act/Pool queues to avoid
sequencer head-of-line blocking.
"""
import sys

sys.path.insert(0, "/opt/trn_rl_repo")

import numpy as np
import ml_dtypes
from contextlib import ExitStack

import concourse.bass as bass
import concourse.tile as tile
import concourse.bacc as bacc
from concourse import mybir
from concourse.bass_utils import run_bass_kernel_spmd

F32 = mybir.dt.float32
BF16 = mybir.dt.bfloat16
AF = mybir.ActivationFunctionType
OP = mybir.AluOpType

B, T, DIM = 4, 4096, 1024
E = 1024
KC = 4
N_CORES = 8
T_LOC = T // 2
TT = 512
NTT = T_LOC // TT   # 4
NE = E // 128       # 8
NK = DIM // 128     # 8

ALPHA2_ON_POOL = True


def _build_kernel(profile_mode=False):
    nc = bacc.Bacc("TRN2", target_bir_lowering=False, debug=False,
                   num_devices=1 if profile_mode else N_CORES)

    xT = nc.dram_tensor("xT", [DIM, T_LOC], BF16, kind="ExternalInput")
    xa_halo = nc.dram_tensor("xa_halo", [E, KC - 1], BF16, kind="ExternalInput")
    w_in_g = nc.dram_tensor("w_in_g", [DIM, E], BF16, kind="ExternalInput")
    w_in_x = nc.dram_tensor("w_in_x", [DIM, E], BF16, kind="ExternalInput")
    w_gates = nc.dram_tensor("w_gates", [E, 2 * E], BF16, kind="ExternalInput")
    w_out = nc.dram_tensor("w_out", [E, DIM], BF16, kind="ExternalInput")
    wc = nc.dram_tensor("wc", [E, KC], F32, kind="ExternalInput")
    b_conv = nc.dram_tensor("b_conv", [E, 1], F32, kind="ExternalInput")
    neg_ch = nc.dram_tensor("neg_ch", [E, 1], F32, kind="ExternalInput")
    b_fh = nc.dram_tensor("b_fh", [E, 1], F32, kind="ExternalInput")
    b_ih = nc.dram_tensor("b_ih", [E, 1], F32, kind="ExternalInput")
    mask_c = nc.dram_tensor("mask_c", [128, 1], F32, kind="ExternalInput")
    mask_u = nc.dram_tensor("mask_u", [128, 1], F32, kind="ExternalInput")
    out = nc.dram_tensor("out", [T_LOC, DIM], BF16, kind="ExternalOutput")

    with tile.TileContext(nc) as tc, ExitStack() as ctx:
        _body(ctx, tc, nc, profile_mode=profile_mode,
              xT=xT, xa_halo=xa_halo, w_in_g=w_in_g, w_in_x=w_in_x,
              w_gates=w_gates, w_out=w_out, wc=wc, b_conv=b_conv,
              neg_ch=neg_ch, b_fh=b_fh, b_ih=b_ih,
              mask_c=mask_c, mask_u=mask_u, out=out)
    nc.compile()
    return nc


def _body(ctx, tc, nc, *, xT, xa_halo, w_in_g, w_in_x, w_gates, w_out, wc,
          b_conv, neg_ch, b_fh, b_ih, mask_c, mask_u, out,
          profile_mode=False):
    consts = ctx.enter_context(tc.tile_pool(name="consts", bufs=1))
    ps = ctx.enter_context(tc.tile_pool(name="ps", bufs=8, space="PSUM"))
    dram = ctx.enter_context(tc.tile_pool(name="dram", bufs=1, space="DRAM"))
    wpool = ctx.enter_context(tc.tile_pool(name="weights", bufs=1, side="right"))
    xs = ctx.enter_context(tc.tile_pool(name="xs", bufs=3))
    xap = ctx.enter_context(tc.tile_pool(name="xap", bufs=2))
    xcp = ctx.enter_context(tc.tile_pool(name="xcp", bufs=2))
    sfp = ctx.enter_context(tc.tile_pool(name="sfp", bufs=9))
    sip = ctx.enter_context(tc.tile_pool(name="sip", bufs=9))
    alp = ctx.enter_context(tc.tile_pool(name="alp", bufs=3))
    a2p = ctx.enter_context(tc.tile_pool(name="a2p", bufs=3))
    bep = ctx.enter_context(tc.tile_pool(name="bep", bufs=3))
    bsp = ctx.enter_context(tc.tile_pool(name="bsp", bufs=2))
    up = ctx.enter_context(tc.tile_pool(name="up", bufs=2))
    spl = ctx.enter_context(tc.tile_pool(name="spl", bufs=2))
    hvp = ctx.enter_context(tc.tile_pool(name="hvp", bufs=2))
    osbp = ctx.enter_context(tc.tile_pool(name="osbp", bufs=2))

    # --- constants (Act queue keeps SP free for weights/x) ---
    def chan_const(t_dram, n):
        t = consts.tile([128, NE, n], F32, tag=t_dram.name, name=t_dram.name)
        nc.scalar.dma_start(t[:], t_dram.ap().rearrange("(m p) n -> p m n", p=128))
        return t

    wc_sb = chan_const(wc, KC)
    bc_sb = chan_const(b_conv, 1)
    nch_sb = chan_const(neg_ch, 1)
    bfh_sb = chan_const(b_fh, 1)
    bih_sb = chan_const(b_ih, 1)
    mc_sb = consts.tile([128, 1], F32, tag="mc")
    nc.scalar.dma_start(mc_sb[:], mask_c.ap()[:])
    mu_sb = consts.tile([128, 1], F32, tag="mu")
    nc.scalar.dma_start(mu_sb[:], mask_u.ap()[:])
    zeros = consts.tile([128, TT], F32, tag="zeros")
    nc.vector.memset(zeros[:], 0.0)
    c_zero = consts.tile([128, 1], F32, tag="c_zero")
    nc.vector.memset(c_zero[:], 0.0)
    c_qb = consts.tile([128, 1], F32, tag="c_qb")
    nc.vector.memset(c_qb[:], 0.25000025)
    hc = consts.tile([128, NE], F32, tag="hc")
    plc = consts.tile([128, NE], F32, tag="plc")
    contrib = consts.tile([128, NE], F32, tag="contrib")
    craw = consts.tile([128, NE], F32, tag="craw")
    carry = consts.tile([128, NE], F32, tag="carry")

    h_dram = dram.tile([NTT, 128, NE, TT], BF16, tag="h_spill")
    p_dram = dram.tile([NTT, 128, NE, TT], BF16, tag="p_spill")
    cc_in = dram.tile([E], F32, tag="cc_in")
    cc_out = dram.tile([E], F32, tag="cc_out")

    # --- weights (persistent bf16; w_out shares the w_in_x slot) ---
    wx_sb = wpool.tile([128, NK, E], BF16, tag="wxo", name="wx_sb")
    wg_sb = wpool.tile([128, NK, 2 * E], BF16, tag="wg", name="wg_sb")
    wgi_sb = wpool.tile([128, NK, E], BF16, tag="wgi", name="wgi_sb")
    wx_src = w_in_x.ap().rearrange("(k p) e -> p k e", p=128)
    wg_src = w_gates.ap().rearrange("(k p) f -> p k f", p=128)
    wgi_src = w_in_g.ap().rearrange("(k p) e -> p k e", p=128)
    wo_src = w_out.ap().rearrange("(k p) c -> p k c", p=128)
    xT_r = xT.ap().rearrange("(k p) t -> p k t", p=128)
    halo_r = xa_halo.ap().rearrange("(m p) n -> p m n", p=128)

    def load_x_tile(tt):
        t = xs.tile([128, NK, TT], BF16, tag="xstream", name="xt")
        nc.sync.dma_start(t[:], xT_r[:, :, tt * TT:(tt + 1) * TT])
        return t

    # ================= pass 1: xa proj + conv + gates ==================
    # The u/scan stage for tile tt runs one tile deferred (during tt+1)
    # so the next tile's conv is first in the DVE queue and PE never
    # waits on the scan tail.
    def deferred_act(st):
        tt, sfs, sis, xcs = st
        als, bes = {}, {}
        for m in range(NE):  # alpha = exp(-c/2*vf - c/2)  [same table as tanh]
            al = alp.tile([128, TT], F32, tag="al", name="al")
            nc.scalar.activation(al[:], sfs[m][:], AF.Exp,
                                 scale=nch_sb[:, m, 0:1],
                                 bias=nch_sb[:, m, 0:1])
            als[m] = al
            a2 = a2p.tile([128, TT], F32, tag="a2", name="a2")
            nc.gpsimd.tensor_mul(a2[:], al[:], al[:])
            bes[m] = a2
        for m in range(NE):  # beta/2 = sqrt(0.25000025 - 0.25*alpha^2)
            be = bep.tile([128, TT], F32, tag="be", name="be")
            nc.scalar.activation(be[:], bes[m][:], AF.Sqrt,
                                 scale=-0.25, bias=c_qb[:])
            bes[m] = be
        return als, bes

    def deferred_dve(st, als, bes):
        tt, sfs, sis, xcs = st
        h_all = spl.tile([128, NE, TT], BF16, tag="hall", name="h_all")
        p_all = spl.tile([128, NE, TT], BF16, tag="pall", name="p_all")
        for m in range(NE):
            bs = bsp.tile([128, TT], F32, tag="bs", name="bs")
            nc.vector.scalar_tensor_tensor(bs[:], sis[m][:], 1.0, bes[m][:],
                                           op0=OP.add, op1=OP.mult)
            u = up.tile([128, TT], F32, tag="u", name="u")
            nc.vector.tensor_mul(u[:], bs[:], xcs[m][:])
            nc.vector.tensor_tensor_scan(
                h_all[:, m], als[m][:], u[:],
                0.0 if tt == 0 else hc[:, m:m + 1],
                op0=OP.mult, op1=OP.add)
            nc.vector.tensor_copy(hc[:, m:m + 1], h_all[:, m, TT - 1:TT])
            nc.vector.tensor_tensor_scan(
                p_all[:, m], als[m][:], zeros[:],
                1.0 if tt == 0 else plc[:, m:m + 1],
                op0=OP.mult, op1=OP.add)
            nc.vector.tensor_copy(plc[:, m:m + 1], p_all[:, m, TT - 1:TT])
        nc.gpsimd.dma_start(h_dram[tt], h_all[:])
        nc.gpsimd.dma_start(p_dram[tt], p_all[:])

    prev_xa = None
    pending = None
    xt_p2 = None
    for tt in range(NTT):
        if tt == 0:
            xt = xs.tile([128, NK, TT], BF16, tag="xstream", name="xt")
            for k in range(NK):
                nc.sync.dma_start(wx_sb[:, k], wx_src[:, k])
                nc.sync.dma_start(xt[:, k], xT_r[:, k, 0:TT])
            for k in range(NK):
                nc.sync.dma_start(wg_sb[:, k], wg_src[:, k])
        else:
            xt = load_x_tile(tt)

        xas, xcs = [], []
        for m in range(NE):
            pa = ps.tile([128, TT], F32, tag="ps", name="pa")
            for k in range(NK):
                nc.tensor.matmul(pa[:], wx_sb[:, k, m * 128:(m + 1) * 128],
                                 xt[:, k], start=(k == 0), stop=(k == NK - 1))
            xa = xap.tile([128, TT + KC - 1], BF16, tag=f"xa{m}", name="xa")
            nc.scalar.copy(xa[:, KC - 1:TT + KC - 1], pa[:])
            if tt == 0:
                nc.scalar.dma_start(xa[:, 0:KC - 1], halo_r[:, m])
            else:
                nc.vector.tensor_copy(xa[:, 0:KC - 1],
                                      prev_xa[m][:, TT:TT + KC - 1])
            xc = xcp.tile([128, TT], BF16, tag=f"xc{m}", name="xc")
            nc.vector.tensor_scalar(
                xc[:], xa[:, 0:TT], wc_sb[:, m, 0:1], bc_sb[:, m, 0:1],
                op0=OP.mult, op1=OP.add)
            for j in range(1, KC):
                nc.vector.scalar_tensor_tensor(
                    xc[:], xa[:, j:j + TT], wc_sb[:, m, j:j + 1],
                    xc[:], op0=OP.mult, op1=OP.add)
            xas.append(xa)
            xcs.append(xc)
        if pending is not None:
            d_als, d_bes = deferred_act(pending)
            deferred_dve(pending, d_als, d_bes)

        sfs, sis = {}, {}
        for g in range(2):
            ms = range(g * 4, g * 4 + 4)
            pfs, pis = {}, {}
            for m in ms:
                pf = ps.tile([128, TT], F32, tag="ps", name="pf")
                for k in range(NK):
                    nc.tensor.matmul(pf[:], wg_sb[:, k, m * 128:(m + 1) * 128],
                                     xcs[k][:], start=(k == 0), stop=(k == NK - 1))
                pfs[m] = pf
                pi = ps.tile([128, TT], F32, tag="ps", name="pi")
                for k in range(NK):
                    nc.tensor.matmul(pi[:], wg_sb[:, k, E + m * 128:E + (m + 1) * 128],
                                     xcs[k][:], start=(k == 0), stop=(k == NK - 1))
                pis[m] = pi
            for m in ms:  # sigmoid(x) = 0.5*tanh(x/2)+0.5, folded downstream
                sf = sfp.tile([128, TT], BF16, tag="sf", name="sf")
                nc.scalar.activation(sf[:], pfs[m][:], AF.Tanh,
                                     scale=0.5, bias=bfh_sb[:, m, 0:1])
                sfs[m] = sf
                si = sip.tile([128, TT], BF16, tag="si", name="si")
                nc.scalar.activation(si[:], pis[m][:], AF.Tanh,
                                     scale=0.5, bias=bih_sb[:, m, 0:1])
                sis[m] = si

        for k in (2 * tt, 2 * tt + 1):
            nc.sync.dma_start(wgi_sb[:, k], wgi_src[:, k])
        prev_xa = xas
        pending = (tt, sfs, sis, xcs)
        if tt == NTT - 1:
            xt_p2 = load_x_tile(0)

    d_als, d_bes = deferred_act(pending)
    deferred_dve(pending, d_als, d_bes)
    pending = None

    # w_out loads into the (now dead) w_in_x slot
    wo_sb = wpool.tile([128, NK, DIM], BF16, tag="wxo", name="wo_sb")
    for k in range(NK):
        nc.sync.dma_start(wo_sb[:, k], wo_src[:, k])

    # ====== pass 2 prefill: first two gate projections (carry-free) ====
    def gate_phase(tt, xt):
        h2 = spl.tile([128, NE, TT], BF16, tag="hall", name="h2")
        nc.sync.dma_start(h2[:], h_dram[tt])
        p2 = spl.tile([128, NE, TT], BF16, tag="pall", name="p2")
        nc.sync.dma_start(p2[:], p_dram[tt])
        ggs = []
        for m in range(NE):
            pg = ps.tile([128, TT], F32, tag="ps", name="pg")
            for k in range(NK):
                nc.tensor.matmul(pg[:], wgi_sb[:, k, m * 128:(m + 1) * 128],
                                 xt[:, k], start=(k == 0), stop=(k == NK - 1))
            gg = xcp.tile([128, TT], BF16, tag=f"xc{m}", name="gg")
            nc.scalar.activation(gg[:], pg[:], AF.Gelu, bias=c_zero[:])
            ggs.append(gg)
        return h2, p2, ggs

    def y_phase(tt, st):
        h2, p2, ggs = st
        yt = []
        for m in range(NE):
            hv = hvp.tile([128, TT], F32, tag="hv", name="hv")
            nc.vector.scalar_tensor_tensor(
                hv[:], p2[:, m], carry[:, m:m + 1], h2[:, m],
                op0=OP.mult, op1=OP.add)
            y = xap.tile([128, TT + KC - 1], BF16, tag=f"xa{m}", name="y")
            nc.vector.tensor_mul(y[:, 0:TT], ggs[m][:], hv[:])
            yt.append(y)
        return yt

    def out_phase(tt, yt):
        for q in range(TT // 128):
            pos = [ps.tile([128, 512], F32, tag="ps", name="po")
                   for _ in range(2)]
            for k in range(NE):
                for n in range(2):
                    nc.tensor.matmul(
                        pos[n][:], yt[k][:, q * 128:(q + 1) * 128],
                        wo_sb[:, k, n * 512:(n + 1) * 512],
                        start=(k == 0), stop=(k == NE - 1))
            osb = osbp.tile([128, DIM], BF16, tag="osb", name="osb")
            for n in range(2):
                nc.scalar.copy(osb[:, n * 512:(n + 1) * 512], pos[n][:])
            nc.scalar.dma_start(
                out.ap()[tt * TT + q * 128:tt * TT + (q + 1) * 128, :],
                osb[:])

    gstates = {}
    gstates[0] = gate_phase(0, xt_p2)
    gstates[1] = gate_phase(1, load_x_tile(1))

    # ================= carry exchange (pairwise AllReduce, 4KB) ========
    nc.vector.tensor_scalar(contrib[:], hc[:], mc_sb[:, 0:1], None,
                            op0=OP.mult)
    nc.sync.dma_start(cc_in[:].rearrange("(j p) -> p j", p=128), contrib[:])
    if profile_mode:
        nc.sync.dma_start(cc_out[:], cc_in[:])
    else:
        nc.gpsimd.collective_compute(
            "AllReduce", OP.add,
            replica_groups=[[0, 1], [2, 3], [4, 5], [6, 7]],
            ins=[cc_in[:].opt()], outs=[cc_out[:].opt()])
    nc.sync.dma_start(craw[:], cc_out[:].rearrange("(j p) -> p j", p=128))
    nc.vector.tensor_scalar(carry[:], craw[:], mu_sb[:, 0:1], None,
                            op0=OP.mult)

    # ================= pass 2: correction + out proj ===================
    for tt in range(NTT):
        yt = y_phase(tt, gstates.pop(tt))
        if tt + 2 < NTT:
            gstates[tt + 2] = gate_phase(tt + 2, load_x_tile(tt + 2))
        out_phase(tt, yt)


_NC_CACHE = {}


def _get_nc():
    if "nc" not in _NC_CACHE:
        _NC_CACHE["nc"] = _build_kernel()
    return _NC_CACHE["nc"]


def _softplus(x):
    return np.logaddexp(0.0, x)


def kernel(x, w_in, w_conv, b_conv, w_gates, b_gates, forget_base, w_out,
           _want_trace=False):
    BF = ml_dtypes.bfloat16
    x = np.asarray(x, dtype=np.float32)
    w_in = np.asarray(w_in, dtype=np.float32)
    w_conv = np.asarray(w_conv, dtype=np.float32)
    b_conv = np.asarray(b_conv, dtype=np.float32)
    w_gates = np.asarray(w_gates, dtype=np.float32)
    b_gates = np.asarray(b_gates, dtype=np.float32)
    forget_base = np.asarray(forget_base, dtype=np.float32)
    w_out = np.asarray(w_out, dtype=np.float32)

    nc = _get_nc()

    w_in_g = np.ascontiguousarray(w_in[:E].T).astype(BF)     # [DIM, E]
    w_in_x = np.ascontiguousarray(w_in[E:].T).astype(BF)     # [DIM, E]
    w_gates_T = np.ascontiguousarray(w_gates.T).astype(BF)   # [E, 2E]
    w_out_T = np.ascontiguousarray(w_out.T).astype(BF)       # [E, DIM]
    wc_r = np.ascontiguousarray(w_conv.reshape(E, KC))
    neg_c = (-8.0 * _softplus(forget_base.astype(np.float64))).astype(
        np.float32)[:, None]

    common = {
        "w_in_g": w_in_g, "w_in_x": w_in_x, "w_gates": w_gates_T,
        "w_out": w_out_T, "wc": wc_r, "b_conv": b_conv[:, None].copy(),
        "neg_ch": 0.5 * neg_c,
        "b_fh": 0.5 * b_gates[:E, None], "b_ih": 0.5 * b_gates[E:, None],
    }
    in_maps = []
    for k in range(N_CORES):
        b, half = k // 2, k % 2
        t0 = half * T_LOC
        xT_loc = np.ascontiguousarray(x[b, t0:t0 + T_LOC, :].T).astype(BF)
        if half == 1:
            xa_halo = (x[b, t0 - (KC - 1):t0, :] @ w_in[E:].T).T
            xa_halo = np.ascontiguousarray(xa_halo).astype(BF)
        else:
            xa_halo = np.zeros((E, KC - 1), dtype=BF)
        mc = np.full((128, 1), 1.0 if half == 0 else 0.0, dtype=np.float32)
        mu = np.full((128, 1), 0.0 if half == 0 else 1.0, dtype=np.float32)
        in_maps.append({**common, "xT": xT_loc, "xa_halo": xa_halo,
                        "mask_c": mc, "mask_u": mu})

    res = run_bass_kernel_spmd(nc, in_maps, core_ids=list(range(N_CORES)),
                               trace=_want_trace)
    out_full = np.empty((B, T, DIM), dtype=np.float32)
    for k in range(N_CORES):
        b, half = k // 2, k % 2
        out_full[b, half * T_LOC:(half + 1) * T_LOC, :] = \
            res.results[k]["out"].astype(np.float32)
    if _want_trace:
        return out_full, res
    return out_full
